# revision 6
# baseline (speedup 1.0000x reference)
"""CorrelationFusion Trainium2 kernel.

Per-clip math (T=8 frames, G=4 groups, 3x3 correlation window):
  corr[g, tt*9+ij, p] = sum_cp x[tt, g*64+cp, p] * xpad[tt+1, g*64+cp, p+d(ij)]
  wx[g, o*8+t, p]     = sum_i conv_w[g, o*8+t, i]*corr[g, i, p] + conv_b[g, o*8+t]
  out[o, c, p]        = sum_t wx'[g, o*8+t, p] * x[t, c, p],  c = cp*4+g
  (wx' = wx + 1 on the t==o rows -- the residual folded into the conv bias)

Mapping (one clip per NeuronCore, 8 cores data-parallel):
  - per-pixel products on DVE in bf16 (2x mode); channels on partitions
  - partition reductions (over cp / over t) via TensorE matmuls into PSUM
  - single replication-padded frame tile per (half, t); dj shifts read at
    odd element offsets (DVE keeps 2x for unaligned bf16 APs)
  - products batched: 3 dj-shifts per DVE op (corr), 2 cpc per op (fusion)
  - wx replicated to the (cpk,t) layout via one broadcast-read DMA per (g,o)
  - corr(half 1) emission interleaved with fusion(pair 0) to keep DVE+PE busy
  - output stored bf16 on device, upcast to fp32 host-side
"""

import numpy as np
import ml_dtypes

T = 8
TO = 8
G = 4
C = 256
H = 56
W = 56
PIX = H * W
NCORES = 8
PH = 58           # padded rows
PWID = 58         # padded cols
PPITCH = PH * PWID
NCH = 7           # pixel chunks
CHN = 448         # pixels per chunk

_CACHE = {}


def _build_module():
    import concourse.bass as bass
    import concourse.bacc as bacc
    import concourse.mybir as mybir
    import concourse.tile as tile

    fp32 = mybir.dt.float32
    bf16 = mybir.dt.bfloat16

    nc = bacc.Bacc(name="corrfusion")
    xin = nc.dram_tensor("xin", [T, C, H, W], bf16, kind="ExternalInput")
    wf2 = nc.dram_tensor("wf2", [128, 2, 128], bf16, kind="ExternalInput")
    bmat = nc.dram_tensor("bmat", [128, 251], bf16, kind="ExternalInput")
    tones = nc.dram_tensor("tones", [128, 8, 128], bf16, kind="ExternalInput")
    bvec = nc.dram_tensor("bvec", [128, 2], fp32, kind="ExternalInput")
    out = nc.dram_tensor("out", [TO, C, H, W], bf16, kind="ExternalOutput")

    xin_flat = xin.rearrange("t c h w -> t c (h w)")
    xin_base = xin[:, :, :, :]
    out_base = out[:, :, :, :]

    with tile.TileContext(nc) as tc:
        with tc.tile_pool(name="consts", bufs=1) as consts, \
             tc.tile_pool(name="corrbuf", bufs=1) as corrbuf, \
             tc.tile_pool(name="frames", bufs=1) as frames, \
             tc.tile_pool(name="stage", bufs=1) as stage, \
             tc.tile_pool(name="prods", bufs=2) as prods, \
             tc.tile_pool(name="xt", bufs=1) as xtp, \
             tc.tile_pool(name="wrep", bufs=2) as wrepp, \
             tc.tile_pool(name="pr2", bufs=4) as pr2p, \
             tc.tile_pool(name="xob", bufs=1) as xobp, \
             tc.tile_pool(name="wxd", bufs=1, space="DRAM") as wxdp:

            wf_sb = consts.tile([128, 2, 128], bf16)
            nc.sync.dma_start(out=wf_sb, in_=wf2[:, :, :])
            bm_sb = consts.tile([128, 251], bf16)
            nc.sync.dma_start(out=bm_sb, in_=bmat[:, :])
            to_sb = consts.tile([128, 8, 128], bf16)
            nc.sync.dma_start(out=to_sb, in_=tones[:, :, :])
            bv_sb = consts.tile([128, 2], fp32)
            nc.sync.dma_start(out=bv_sb, in_=bvec[:, :])

            corr_sb = [
                corrbuf.tile([128, PIX], bf16, tag=f"corr{i}", name=f"corr{i}")
                for i in range(2)
            ]
            # wx reuses the corr buffer of its pair: conv reads corr chunk c
            # strictly before the drain writes wx chunk c (disjoint regions
            # are tracked, same-region access is read-then-write in order)
            wx_sb = corr_sb
            for i in range(2):
                nc.vector.memset(corr_sb[i][96:128, :], 0.0)

            ptile = {}

            def load_frame(ct, t):
                stg = stage.tile([128, PIX], bf16, tag="fstage", name=f"stg{ct}_{t}")
                nc.sync.dma_start(out=stg, in_=xin_flat[t, ct * 128:(ct + 1) * 128, :])
                stg3 = stg.rearrange("p (h w) -> p h w", h=H)
                P = frames.tile([128, PH, PWID], bf16, tag=f"P{t % 4}", name=f"P{ct}_{t}")
                nc.scalar.copy(P[:, 1:57, 1:57], stg3)
                # replication pads: cols first, then full-width rows (corners ok)
                nc.vector.tensor_copy(P[:, 1:57, 0:1], P[:, 1:57, 1:2])
                nc.vector.tensor_copy(P[:, 1:57, 57:58], P[:, 1:57, 56:57])
                nc.vector.tensor_copy(P[:, 0:1, 0:58], P[:, 1:2, 0:58])
                nc.vector.tensor_copy(P[:, 57:58, 0:58], P[:, 56:57, 0:58])
                ptile[(ct, t)] = P

            def corr_unit(ct, tt, di, cps):
                """One DVE product (3 dj shifts batched) + 21 PE passes."""
                Pa = ptile[(ct, tt)]
                Pb = ptile[(ct, tt + 1)]
                pr = prods.tile([128, 3, PIX], bf16, tag="pr", name=f"pr{ct}_{tt}_{di}")
                a_ap = bass.AP(
                    tensor=Pa.tensor,
                    offset=Pa.offset + 1 * PWID + 1,
                    ap=[[PPITCH, 128], [0, 3], [PWID, 56], [1, 56]],
                )
                b_ap = bass.AP(
                    tensor=Pb.tensor,
                    offset=Pb.offset + (1 + di) * PWID + 0,
                    ap=[[PPITCH, 128], [1, 3], [PWID, 56], [1, 56]],
                )
                pr_view = bass.AP(
                    tensor=pr.tensor,
                    offset=pr.offset,
                    ap=[[3 * PIX, 128], [PIX, 3], [W, 56], [1, 56]],
                )
                nc.vector.tensor_mul(pr_view, a_ap, b_ap)
                for dj in range(3):
                    r = tt * 9 + (di + 1) * 3 + dj
                    lhsT = bm_sb[:, 125 - r:251 - r]
                    for c in range(NCH):
                        nc.tensor.matmul(
                            cps[c],
                            lhsT,
                            pr[:, dj, c * CHN:(c + 1) * CHN],
                            start=(r == 0),
                            stop=(r == 62),
                        )

            def corr_drain(ct, cps):
                for c in range(NCH):
                    nc.scalar.copy(
                        corr_sb[ct][0:126, c * CHN:(c + 1) * CHN], cps[c]
                    )

            def conv(gp, psum):
                """Grouped 1x1 conv: wx = wf2[gp].T @ corr (+bias at drain)."""
                wd = wxdp.tile([128, PIX], bf16, tag=f"wxd{gp}", name=f"wxd{gp}")
                for c in range(NCH):
                    wpp = psum.tile([128, CHN], fp32, tag="sp", name=f"wpp{gp}_{c}")
                    nc.tensor.matmul(
                        wpp,
                        wf_sb[:, gp, :],
                        corr_sb[gp][:, c * CHN:(c + 1) * CHN],
                        start=True,
                        stop=True,
                    )
                    nc.scalar.activation(
                        wx_sb[gp][:, c * CHN:(c + 1) * CHN],
                        wpp,
                        mybir.ActivationFunctionType.Identity,
                        bias=bv_sb[:, gp:gp + 1],
                        scale=1.0,
                    )
                nc.sync.dma_start(out=wd, in_=wx_sb[gp])
                return wd

            def load_xt(g):
                xt = xtp.tile([128, 4, PIX], bf16, tag=f"xt{g % 2}", name=f"xt{g}")
                for cpc in range(4):
                    src = bass.AP(
                        tensor=xin_base.tensor,
                        offset=(cpc * 64 + g) * PIX,
                        ap=[[4 * PIX, 16], [C * PIX, T], [1, PIX]],
                    )
                    nc.sync.dma_start(out=xt[:, cpc, :], in_=src)
                return xt

            def fuse_unit(gp, o, wxd, xts, psum):
                """Weighted frame-sum for one (group-pair, output frame)."""
                pr2s = {}
                for gh in range(2):
                    wrep = wrepp.tile([128, PIX], bf16, tag="wrep", name=f"wr{gp}_{o}_{gh}")
                    wsrc = bass.AP(
                        tensor=wxd.tensor,
                        offset=wxd.offset + (gh * 64 + o * 8) * PIX,
                        ap=[[0, 16], [PIX, 8], [1, PIX]],
                    )
                    nc.sync.dma_start(out=wrep, in_=wsrc)
                    wr_bc = bass.AP(
                        tensor=wrep.tensor,
                        offset=wrep.offset,
                        ap=[[PIX, 128], [0, 2], [1, PIX]],
                    )
                    for j in range(2):
                        pr2 = pr2p.tile(
                            [128, 2, PIX], bf16, tag="pr2", name=f"p2_{gp}_{o}_{gh}_{j}"
                        )
                        nc.vector.tensor_mul(pr2, xts[gh][:, 2 * j:2 * j + 2, :], wr_bc)
                        pr2s[(gh, j)] = pr2
                xout = xobp.tile([128, PIX], bf16, tag="xout", name=f"xo{gp}_{o}")
                for c in range(NCH):
                    xop = psum.tile([128, CHN], fp32, tag="sp", name=f"xop{gp}_{o}_{c}")
                    for s in range(8):
                        gh, cpc = s // 4, s % 4
                        nc.tensor.matmul(
                            xop,
                            to_sb[:, s, :],
                            pr2s[(gh, cpc // 2)][:, cpc % 2, c * CHN:(c + 1) * CHN],
                            start=(s == 0),
                            stop=(s == 7),
                        )
                    nc.scalar.copy(xout[:, c * CHN:(c + 1) * CHN], xop)
                for gh in range(2):
                    dst = bass.AP(
                        tensor=out_base.tensor,
                        offset=(o * C + gp * 2 + gh) * PIX,
                        ap=[[64 * PIX, 4], [4 * PIX, 16], [1, PIX]],
                    )
                    nc.sync.dma_start(out=dst, in_=xout[gh * 64:(gh + 1) * 64, :])

            units = [(tt, di) for tt in range(T - 1) for di in (-1, 0, 1)]

            # ---------------- emission schedule ----------------
            with tc.tile_pool(name="cpsum", bufs=1, space="PSUM") as cpsum, \
                 tc.tile_pool(name="spsum", bufs=1, space="PSUM") as spsum:
                cps = [
                    cpsum.tile([126, CHN], fp32, tag=f"cp{c}", name=f"cps{c}")
                    for c in range(NCH)
                ]
                # half 0 frames + corr
                for t in range(2):
                    load_frame(0, t)
                for k, (tt, di) in enumerate(units):
                    if di == -1 and tt + 2 < T:
                        load_frame(0, tt + 2)
                    corr_unit(0, tt, di, cps)
                corr_drain(0, cps)

                # conv pair 0 + xt loads for its groups
                wxd0 = conv(0, spsum)
                xts0 = [load_xt(0), load_xt(1)]

                # half 1 frames + corr, interleaved with fusion for pair 0
                for t in range(2):
                    load_frame(1, t)
                fuse_at = {2: 0, 5: 1, 7: 2, 10: 3, 13: 4, 15: 5, 18: 6, 20: 7}
                for k, (tt, di) in enumerate(units):
                    if di == -1 and tt + 2 < T:
                        load_frame(1, tt + 2)
                    corr_unit(1, tt, di, cps)
                    if k in fuse_at:
                        fuse_unit(0, fuse_at[k], wxd0, xts0, spsum)
                corr_drain(1, cps)

            # pair-1 tail: corr PSUM banks are free, use a double-buffered pool
            with tc.tile_pool(name="spsum2", bufs=2, space="PSUM") as spsum2:
                wxd1 = conv(1, spsum2)
                xts1 = [load_xt(2), load_xt(3)]
                for o in range(TO):
                    fuse_unit(1, o, wxd1, xts1, spsum2)

    nc.compile()
    return nc


def _get_module():
    if "nc" not in _CACHE:
        _CACHE["nc"] = _build_module()
    return _CACHE["nc"]


def _consts(conv_w, conv_b):
    conv_w = np.asarray(conv_w, np.float32)
    conv_b = np.asarray(conv_b, np.float32)
    # block-diagonal fused conv weights per group-pair:
    #   wf2[k, gp, m]; m = gh*64 + o*8 + t; k rows gh*63..+63 hold
    #   conv_w[gp*2+gh, o*8+t, :].  Bias (+1.0 residual when t==o) applied at
    #   the PSUM drain as a per-partition activation bias (bvec).
    wf2 = np.zeros((128, 2, 128), np.float32)
    bvec = np.zeros((128, 2), np.float32)
    for gp in range(2):
        for gh in range(2):
            g = gp * 2 + gh
            half = gh * 63
            for o in range(TO):
                for t in range(T):
                    m = gh * 64 + o * 8 + t
                    wf2[half:half + 63, gp, m] = conv_w[g, o * 8 + t]
                    bvec[m, gp] = conv_b[g, o * 8 + t] + (1.0 if t == o else 0.0)

    # corr-reduce matrix: sliding window puts product r's group-sums into
    # PSUM rows r (channels 0-63) and 63+r (channels 64-127)
    bm = np.zeros((128, 251), np.float32)
    bm[0:64, 125] = 1.0
    bm[64:128, 188] = 1.0

    # t-reduce ones: to[p=(cpk,t), s=(gh,cpc), m] = 1 iff m == gh*64+cpc*16+cpk
    to = np.zeros((128, 8, 128), np.float32)
    for s in range(8):
        gh, cpc = s // 4, s % 4
        for cpk in range(16):
            to[cpk * 8:(cpk + 1) * 8, s, gh * 64 + cpc * 16 + cpk] = 1.0

    return (
        wf2.astype(ml_dtypes.bfloat16),
        bm.astype(ml_dtypes.bfloat16),
        to.astype(ml_dtypes.bfloat16),
        bvec,
    )


def kernel(x, conv_w, conv_b):
    from concourse.bass_utils import run_bass_kernel_spmd

    nc = _get_module()
    wf, bm, to, bv = _consts(conv_w, conv_b)
    x = np.asarray(x, np.float32).astype(ml_dtypes.bfloat16)
    x8 = np.ascontiguousarray(x.reshape(NCORES, T, C, H, W))
    in_maps = [
        {
            "xin": np.ascontiguousarray(x8[i]),
            "wf2": wf,
            "bmat": bm,
            "tones": to,
            "bvec": bv,
        }
        for i in range(NCORES)
    ]
    res = run_bass_kernel_spmd(nc, in_maps, core_ids=list(range(NCORES)))
    outs = [r["out"].astype(np.float32) for r in res.results]
    return np.concatenate(outs, axis=0)


# revision 7
# speedup vs baseline: 1.1041x; 1.1041x over previous
"""CorrelationFusion Trainium2 kernel.

Per-clip math (T=8 frames, G=4 groups, 3x3 correlation window):
  corr[g, tt*9+ij, p] = sum_cp x[tt, g*64+cp, p] * xpad[tt+1, g*64+cp, p+d(ij)]
  wx[g, o*8+t, p]     = sum_i conv_w[g, o*8+t, i]*corr[g, i, p] + conv_b[g, o*8+t]
  out[o, c, p]        = sum_t wx'[g, o*8+t, p] * x[t, c, p],  c = cp*4+g
  (wx' = wx + 1 on the t==o rows -- the residual folded into the conv bias)

Mapping (one clip per NeuronCore, 8 cores data-parallel):
  - per-pixel products on DVE in bf16 (2x mode); channels on partitions
  - partition reductions (over cp / over t) via TensorE matmuls into PSUM
  - single replication-padded frame tile per (half, t); dj shifts read at
    odd element offsets (DVE keeps 2x for unaligned bf16 APs)
  - products batched: 3 dj-shifts per DVE op (corr), 2 cpc per op (fusion)
  - wx replicated to the (cpk,t) layout via one broadcast-read DMA per (g,o)
  - corr(half 1) emission interleaved with fusion(pair 0) to keep DVE+PE busy
  - output stored bf16 on device, upcast to fp32 host-side
"""

import numpy as np
import ml_dtypes

T = 8
TO = 8
G = 4
C = 256
H = 56
W = 56
PIX = H * W
NCORES = 8
PH = 58           # padded rows
PWID = 58         # padded cols
PPITCH = PH * PWID
NCH = 7           # pixel chunks
CHN = 448         # pixels per chunk

_CACHE = {}


def _build_module():
    import concourse.bass as bass
    import concourse.bacc as bacc
    import concourse.mybir as mybir
    import concourse.tile as tile

    fp32 = mybir.dt.float32
    bf16 = mybir.dt.bfloat16

    nc = bacc.Bacc(name="corrfusion")
    xin = nc.dram_tensor("xin", [T, C, H, W], bf16, kind="ExternalInput")
    wf2 = nc.dram_tensor("wf2", [128, 2, 128], bf16, kind="ExternalInput")
    bmat = nc.dram_tensor("bmat", [128, 251], bf16, kind="ExternalInput")
    tones = nc.dram_tensor("tones", [128, 8, 128], bf16, kind="ExternalInput")
    bvec = nc.dram_tensor("bvec", [128, 2], fp32, kind="ExternalInput")
    out = nc.dram_tensor("out", [TO, C, H, W], bf16, kind="ExternalOutput")

    xin_flat = xin.rearrange("t c h w -> t c (h w)")
    xin_base = xin[:, :, :, :]
    out_base = out[:, :, :, :]

    with tile.TileContext(nc) as tc:
        with tc.tile_pool(name="consts", bufs=1) as consts, \
             tc.tile_pool(name="corrbuf", bufs=1) as corrbuf, \
             tc.tile_pool(name="frames", bufs=1) as frames, \
             tc.tile_pool(name="stage", bufs=1) as stage, \
             tc.tile_pool(name="prods", bufs=2) as prods, \
             tc.tile_pool(name="xt", bufs=1) as xtp, \
             tc.tile_pool(name="wrep", bufs=2) as wrepp, \
             tc.tile_pool(name="pr2", bufs=4) as pr2p, \
             tc.tile_pool(name="xob", bufs=1) as xobp, \
             tc.tile_pool(name="wxd", bufs=1, space="DRAM") as wxdp:

            wf_sb = consts.tile([128, 2, 128], bf16)
            nc.sync.dma_start(out=wf_sb, in_=wf2[:, :, :])
            bm_sb = consts.tile([128, 251], bf16)
            nc.sync.dma_start(out=bm_sb, in_=bmat[:, :])
            to_sb = consts.tile([128, 8, 128], bf16)
            nc.sync.dma_start(out=to_sb, in_=tones[:, :, :])
            bv_sb = consts.tile([128, 2], fp32)
            nc.sync.dma_start(out=bv_sb, in_=bvec[:, :])

            corr_sb = [
                corrbuf.tile([128, PIX], bf16, tag=f"corr{i}", name=f"corr{i}")
                for i in range(2)
            ]
            # wx reuses the corr buffer of its pair: conv reads corr chunk c
            # strictly before the drain writes wx chunk c (disjoint regions
            # are tracked, same-region access is read-then-write in order)
            wx_sb = corr_sb
            for i in range(2):
                nc.vector.memset(corr_sb[i][96:128, :], 0.0)

            ptile = {}

            def load_frame(ct, t):
                stg = stage.tile([128, PIX], bf16, tag="fstage", name=f"stg{ct}_{t}")
                nc.sync.dma_start(out=stg, in_=xin_flat[t, ct * 128:(ct + 1) * 128, :])
                stg3 = stg.rearrange("p (h w) -> p h w", h=H)
                P = frames.tile([128, PH, PWID], bf16, tag=f"P{t % 4}", name=f"P{ct}_{t}")
                nc.scalar.copy(P[:, 1:57, 1:57], stg3)
                # replication pads: cols first, then full-width rows (corners ok)
                nc.vector.tensor_copy(P[:, 1:57, 0:1], P[:, 1:57, 1:2])
                nc.vector.tensor_copy(P[:, 1:57, 57:58], P[:, 1:57, 56:57])
                nc.vector.tensor_copy(P[:, 0:1, 0:58], P[:, 1:2, 0:58])
                nc.vector.tensor_copy(P[:, 57:58, 0:58], P[:, 56:57, 0:58])
                ptile[(ct, t)] = P

            def corr_unit(ct, tt, di, cps):
                """One DVE product (3 dj shifts batched) + 21 PE passes."""
                Pa = ptile[(ct, tt)]
                Pb = ptile[(ct, tt + 1)]
                pr = prods.tile([128, 3, PIX], bf16, tag="pr", name=f"pr{ct}_{tt}_{di}")
                a_ap = bass.AP(
                    tensor=Pa.tensor,
                    offset=Pa.offset + 1 * PWID + 1,
                    ap=[[PPITCH, 128], [0, 3], [PWID, 56], [1, 56]],
                )
                b_ap = bass.AP(
                    tensor=Pb.tensor,
                    offset=Pb.offset + (1 + di) * PWID + 0,
                    ap=[[PPITCH, 128], [1, 3], [PWID, 56], [1, 56]],
                )
                pr_view = bass.AP(
                    tensor=pr.tensor,
                    offset=pr.offset,
                    ap=[[3 * PIX, 128], [PIX, 3], [W, 56], [1, 56]],
                )
                nc.vector.tensor_mul(pr_view, a_ap, b_ap)
                for dj in range(3):
                    r = tt * 9 + (di + 1) * 3 + dj
                    lhsT = bm_sb[:, 125 - r:251 - r]
                    for c in range(NCH):
                        nc.tensor.matmul(
                            cps[c],
                            lhsT,
                            pr[:, dj, c * CHN:(c + 1) * CHN],
                            start=(r == 0),
                            stop=(r == 62),
                        )

            def corr_drain(ct, cps):
                for c in range(NCH):
                    nc.scalar.copy(
                        corr_sb[ct][0:126, c * CHN:(c + 1) * CHN], cps[c]
                    )

            def conv(gp, psum):
                """Grouped 1x1 conv: wx = wf2[gp].T @ corr (+bias at drain)."""
                wd = wxdp.tile([128, PIX], bf16, tag=f"wxd{gp}", name=f"wxd{gp}")
                for c in range(NCH):
                    wpp = psum.tile([128, CHN], fp32, tag="sp", name=f"wpp{gp}_{c}")
                    nc.tensor.matmul(
                        wpp,
                        wf_sb[:, gp, :],
                        corr_sb[gp][:, c * CHN:(c + 1) * CHN],
                        start=True,
                        stop=True,
                    )
                    nc.scalar.activation(
                        wx_sb[gp][:, c * CHN:(c + 1) * CHN],
                        wpp,
                        mybir.ActivationFunctionType.Identity,
                        bias=bv_sb[:, gp:gp + 1],
                        scale=1.0,
                    )
                nc.sync.dma_start(out=wd, in_=wx_sb[gp])
                return wd

            def load_xt(g):
                xt = xtp.tile([128, 4, PIX], bf16, tag=f"xt{g % 2}", name=f"xt{g}")
                for cpc in range(4):
                    src = bass.AP(
                        tensor=xin_base.tensor,
                        offset=(cpc * 64 + g) * PIX,
                        ap=[[4 * PIX, 16], [C * PIX, T], [1, PIX]],
                    )
                    nc.sync.dma_start(out=xt[:, cpc, :], in_=src)
                return xt

            def fuse_unit(gp, o, wxd, xts, psum):
                """Weighted frame-sum for one (group-pair, output frame)."""
                pr2s = {}
                for gh in range(2):
                    wrep = wrepp.tile([128, PIX], bf16, tag="wrep", name=f"wr{gp}_{o}_{gh}")
                    wsrc = bass.AP(
                        tensor=wxd.tensor,
                        offset=wxd.offset + (gh * 64 + o * 8) * PIX,
                        ap=[[0, 16], [PIX, 8], [1, PIX]],
                    )
                    nc.sync.dma_start(out=wrep, in_=wsrc)
                    wr_bc = bass.AP(
                        tensor=wrep.tensor,
                        offset=wrep.offset,
                        ap=[[PIX, 128], [0, 2], [1, PIX]],
                    )
                    for j in range(2):
                        pr2 = pr2p.tile(
                            [128, 2, PIX], bf16, tag="pr2", name=f"p2_{gp}_{o}_{gh}_{j}"
                        )
                        nc.vector.tensor_mul(pr2, xts[gh][:, 2 * j:2 * j + 2, :], wr_bc)
                        pr2s[(gh, j)] = pr2
                xout = xobp.tile([128, PIX], bf16, tag="xout", name=f"xo{gp}_{o}")
                for c in range(NCH):
                    xop = psum.tile([128, CHN], fp32, tag="sp", name=f"xop{gp}_{o}_{c}")
                    for s in range(8):
                        gh, cpc = s // 4, s % 4
                        nc.tensor.matmul(
                            xop,
                            to_sb[:, s, :],
                            pr2s[(gh, cpc // 2)][:, cpc % 2, c * CHN:(c + 1) * CHN],
                            start=(s == 0),
                            stop=(s == 7),
                        )
                    nc.scalar.copy(xout[:, c * CHN:(c + 1) * CHN], xop)
                for gh in range(2):
                    # output channels are group-major: c = g*64 + cpc*16 + cpk
                    dst = bass.AP(
                        tensor=out_base.tensor,
                        offset=(o * C + (gp * 2 + gh) * 64) * PIX,
                        ap=[[PIX, 64], [1, PIX]],
                    )
                    nc.sync.dma_start(out=dst, in_=xout[gh * 64:(gh + 1) * 64, :])

            units = [(tt, di) for tt in range(T - 1) for di in (-1, 0, 1)]

            # ---------------- emission schedule ----------------
            with tc.tile_pool(name="cpsum", bufs=1, space="PSUM") as cpsum, \
                 tc.tile_pool(name="spsum", bufs=1, space="PSUM") as spsum:
                cps = [
                    cpsum.tile([126, CHN], fp32, tag=f"cp{c}", name=f"cps{c}")
                    for c in range(NCH)
                ]
                # half 0 frames + corr
                for t in range(2):
                    load_frame(0, t)
                for k, (tt, di) in enumerate(units):
                    if di == -1 and tt + 2 < T:
                        load_frame(0, tt + 2)
                    corr_unit(0, tt, di, cps)
                corr_drain(0, cps)

                # conv pair 0 + xt loads for its groups
                wxd0 = conv(0, spsum)
                xts0 = [load_xt(0), load_xt(1)]

                # half 1 frames + corr, interleaved with fusion for pair 0
                for t in range(2):
                    load_frame(1, t)
                fuse_at = {2: 0, 5: 1, 7: 2, 10: 3, 13: 4, 15: 5, 18: 6, 20: 7}
                for k, (tt, di) in enumerate(units):
                    if di == -1 and tt + 2 < T:
                        load_frame(1, tt + 2)
                    corr_unit(1, tt, di, cps)
                    if k in fuse_at:
                        fuse_unit(0, fuse_at[k], wxd0, xts0, spsum)
                corr_drain(1, cps)

            # pair-1 tail: corr PSUM banks are free, use a double-buffered pool
            with tc.tile_pool(name="spsum2", bufs=2, space="PSUM") as spsum2:
                wxd1 = conv(1, spsum2)
                xts1 = [load_xt(2), load_xt(3)]
                for o in range(TO):
                    fuse_unit(1, o, wxd1, xts1, spsum2)

    nc.compile()
    return nc


def _get_module():
    if "nc" not in _CACHE:
        _CACHE["nc"] = _build_module()
    return _CACHE["nc"]


def _consts(conv_w, conv_b):
    conv_w = np.asarray(conv_w, np.float32)
    conv_b = np.asarray(conv_b, np.float32)
    # block-diagonal fused conv weights per group-pair:
    #   wf2[k, gp, m]; m = gh*64 + o*8 + t; k rows gh*63..+63 hold
    #   conv_w[gp*2+gh, o*8+t, :].  Bias (+1.0 residual when t==o) applied at
    #   the PSUM drain as a per-partition activation bias (bvec).
    wf2 = np.zeros((128, 2, 128), np.float32)
    bvec = np.zeros((128, 2), np.float32)
    for gp in range(2):
        for gh in range(2):
            g = gp * 2 + gh
            half = gh * 63
            for o in range(TO):
                for t in range(T):
                    m = gh * 64 + o * 8 + t
                    wf2[half:half + 63, gp, m] = conv_w[g, o * 8 + t]
                    bvec[m, gp] = conv_b[g, o * 8 + t] + (1.0 if t == o else 0.0)

    # corr-reduce matrix: sliding window puts product r's group-sums into
    # PSUM rows r (channels 0-63) and 63+r (channels 64-127)
    bm = np.zeros((128, 251), np.float32)
    bm[0:64, 125] = 1.0
    bm[64:128, 188] = 1.0

    # t-reduce ones: to[p=(cpk,t), s=(gh,cpc), m] = 1 iff m == gh*64+cpc*16+cpk
    to = np.zeros((128, 8, 128), np.float32)
    for s in range(8):
        gh, cpc = s // 4, s % 4
        for cpk in range(16):
            to[cpk * 8:(cpk + 1) * 8, s, gh * 64 + cpc * 16 + cpk] = 1.0

    return (
        wf2.astype(ml_dtypes.bfloat16),
        bm.astype(ml_dtypes.bfloat16),
        to.astype(ml_dtypes.bfloat16),
        bvec,
    )


def kernel(x, conv_w, conv_b):
    from concourse.bass_utils import run_bass_kernel_spmd

    nc = _get_module()
    wf, bm, to, bv = _consts(conv_w, conv_b)
    x = np.asarray(x, np.float32).astype(ml_dtypes.bfloat16)
    x8 = np.ascontiguousarray(x.reshape(NCORES, T, C, H, W))
    in_maps = [
        {
            "xin": np.ascontiguousarray(x8[i]),
            "wf2": wf,
            "bmat": bm,
            "tones": to,
            "bvec": bv,
        }
        for i in range(NCORES)
    ]
    res = run_bass_kernel_spmd(nc, in_maps, core_ids=list(range(NCORES)))
    outs = [r["out"].astype(np.float32) for r in res.results]
    return np.concatenate(outs, axis=0)


# revision 12
# speedup vs baseline: 1.2207x; 1.1057x over previous
"""CorrelationFusion Trainium2 kernel.

Per-clip math (T=8 frames, G=4 groups, 3x3 correlation window):
  corr[g, tt*9+ij, p] = sum_cp x[tt, g*64+cp, p] * xpad[tt+1, g*64+cp, p+d(ij)]
  wx[g, o*8+t, p]     = sum_i conv_w[g, o*8+t, i]*corr[g, i, p] + conv_b[g, o*8+t]
  out[o, c, p]        = sum_t wx'[g, o*8+t, p] * x[t, c, p],  c = cp*4+g
  (wx' = wx + 1 on the t==o rows -- the residual folded into the conv bias)

Mapping (one clip per NeuronCore, 8 cores data-parallel):
  - per-pixel products on DVE in bf16 (2x mode); channels on partitions
  - partition reductions (over cp / over t) via TensorE matmuls into PSUM
  - single replication-padded frame tile per (half, t); dj shifts read at
    odd element offsets (DVE keeps 2x for unaligned bf16 APs)
  - products batched: 3 dj-shifts per DVE op (corr), 2 cpc per op (fusion)
  - wx replicated to the (cpk,t) layout via one broadcast-read DMA per (g,o)
  - corr(half 1) emission interleaved with fusion(pair 0) to keep DVE+PE busy
  - output stored bf16 on device, upcast to fp32 host-side
"""

import numpy as np
import ml_dtypes

T = 8
TO = 8
G = 4
C = 256
H = 56
W = 56
PIX = H * W
NCORES = 8
PH = 58           # padded rows
PWID = 58         # padded cols
PPITCH = PH * PWID
NCH = 7           # pixel chunks
CHN = 448         # pixels per chunk

_CACHE = {}


def _build_module():
    import concourse.bass as bass
    import concourse.bacc as bacc
    import concourse.mybir as mybir
    import concourse.tile as tile

    fp32 = mybir.dt.float32
    bf16 = mybir.dt.bfloat16

    nc = bacc.Bacc(name="corrfusion")
    xin = nc.dram_tensor("xin", [T, C, H, W], bf16, kind="ExternalInput")
    wf2 = nc.dram_tensor("wf2", [128, 2, 128], bf16, kind="ExternalInput")
    bmat = nc.dram_tensor("bmat", [128, 251], bf16, kind="ExternalInput")
    tones = nc.dram_tensor("tones", [128, 8, 128], bf16, kind="ExternalInput")
    bvec = nc.dram_tensor("bvec", [128, 2], fp32, kind="ExternalInput")
    out = nc.dram_tensor("out", [TO, C, H, W], bf16, kind="ExternalOutput")

    xin_flat = xin.rearrange("t c h w -> t c (h w)")
    xin_base = xin[:, :, :, :]
    out_base = out[:, :, :, :]

    with tile.TileContext(nc) as tc:
        with tc.tile_pool(name="consts", bufs=1) as consts, \
             tc.tile_pool(name="corrbuf", bufs=1) as corrbuf, \
             tc.tile_pool(name="xt", bufs=1) as xtp, \
             tc.tile_pool(name="wrep", bufs=2) as wrepp, \
             tc.tile_pool(name="xob", bufs=2) as xobp, \
             tc.tile_pool(name="wxd", bufs=1, space="DRAM") as wxdp:

            wf_sb = consts.tile([128, 2, 128], bf16)
            nc.sync.dma_start(out=wf_sb, in_=wf2[:, :, :])
            bm_sb = consts.tile([128, 251], bf16)
            nc.sync.dma_start(out=bm_sb, in_=bmat[:, :])
            to_sb = consts.tile([128, 8, 128], bf16)
            nc.sync.dma_start(out=to_sb, in_=tones[:, :, :])
            bv_sb = consts.tile([128, 2], fp32)
            nc.sync.dma_start(out=bv_sb, in_=bvec[:, :])

            corr_sb = [
                corrbuf.tile([128, PIX], bf16, tag=f"corr{i}", name=f"corr{i}")
                for i in range(2)
            ]
            # wx reuses the corr buffer of its pair: conv reads corr chunk c
            # strictly before the drain writes wx chunk c (disjoint regions
            # are tracked, same-region access is read-then-write in order)
            wx_sb = corr_sb
            for i in range(2):
                nc.vector.memset(corr_sb[i][96:128, :], 0.0)

            ptile = {}

            def load_frame(ct, t, frames, stage):
                stg = stage.tile([128, PIX], bf16, tag="fstage", name=f"stg{ct}_{t}")
                nc.sync.dma_start(out=stg, in_=xin_flat[t, ct * 128:(ct + 1) * 128, :])
                stg3 = stg.rearrange("p (h w) -> p h w", h=H)
                P = frames.tile([128, PH, PWID], bf16, tag=f"P{t % 4}", name=f"P{ct}_{t}")
                nc.scalar.copy(P[:, 1:57, 1:57], stg3)
                # replication pads: cols first, then full-width rows (corners ok)
                nc.vector.tensor_copy(P[:, 1:57, 0:1], P[:, 1:57, 1:2])
                nc.vector.tensor_copy(P[:, 1:57, 57:58], P[:, 1:57, 56:57])
                nc.vector.tensor_copy(P[:, 0:1, 0:58], P[:, 1:2, 0:58])
                nc.vector.tensor_copy(P[:, 57:58, 0:58], P[:, 56:57, 0:58])
                ptile[(ct, t)] = P

            def corr_unit(ct, tt, di, cps, prods):
                """One DVE product (3 dj shifts batched) + 21 PE passes."""
                Pa = ptile[(ct, tt)]
                Pb = ptile[(ct, tt + 1)]
                pr = prods.tile([128, 3, PIX], bf16, tag="pr", name=f"pr{ct}_{tt}_{di}")
                a_ap = bass.AP(
                    tensor=Pa.tensor,
                    offset=Pa.offset + 1 * PWID + 1,
                    ap=[[PPITCH, 128], [0, 3], [PWID, 56], [1, 56]],
                )
                b_ap = bass.AP(
                    tensor=Pb.tensor,
                    offset=Pb.offset + (1 + di) * PWID + 0,
                    ap=[[PPITCH, 128], [1, 3], [PWID, 56], [1, 56]],
                )
                pr_view = bass.AP(
                    tensor=pr.tensor,
                    offset=pr.offset,
                    ap=[[3 * PIX, 128], [PIX, 3], [W, 56], [1, 56]],
                )
                nc.vector.tensor_mul(pr_view, a_ap, b_ap)
                for dj in range(3):
                    r = tt * 9 + (di + 1) * 3 + dj
                    lhsT = bm_sb[:, 125 - r:251 - r]
                    for c in range(NCH):
                        nc.tensor.matmul(
                            cps[c],
                            lhsT,
                            pr[:, dj, c * CHN:(c + 1) * CHN],
                            start=(r == 0),
                            stop=(r == 62),
                        )

            def corr_drain(ct, cps):
                for c in range(NCH):
                    nc.scalar.copy(
                        corr_sb[ct][0:126, c * CHN:(c + 1) * CHN], cps[c]
                    )

            def conv(gp, psum):
                """Grouped 1x1 conv: wx = wf2[gp].T @ corr (+bias at drain)."""
                wd = wxdp.tile([128, PIX], bf16, tag=f"wxd{gp}", name=f"wxd{gp}")
                for c in range(NCH):
                    wpp = psum.tile([128, CHN], fp32, tag="sp", name=f"wpp{gp}_{c}")
                    nc.tensor.matmul(
                        wpp,
                        wf_sb[:, gp, :],
                        corr_sb[gp][:, c * CHN:(c + 1) * CHN],
                        start=True,
                        stop=True,
                    )
                    nc.scalar.activation(
                        wx_sb[gp][:, c * CHN:(c + 1) * CHN],
                        wpp,
                        mybir.ActivationFunctionType.Identity,
                        bias=bv_sb[:, gp:gp + 1],
                        scale=1.0,
                    )
                nc.sync.dma_start(out=wd, in_=wx_sb[gp])
                return wd

            def load_xt(g):
                xt = xtp.tile([128, 4, PIX], bf16, tag=f"xt{g % 2}", name=f"xt{g}")
                for cpc in range(4):
                    src = bass.AP(
                        tensor=xin_base.tensor,
                        offset=(cpc * 64 + g) * PIX,
                        ap=[[4 * PIX, 16], [C * PIX, T], [1, PIX]],
                    )
                    nc.sync.dma_start(out=xt[:, cpc, :], in_=src)
                return xt

            def fuse_unit(gp, o, wxd, xts, psum, prpool, slot_outer=False):
                """Weighted frame-sum for one (group-pair, output frame).

                slot_outer=True holds each to_sb weight set across all 7
                chunk passes (needs 7 PSUM banks; only after corr closes)."""
                pr2s = {}
                for gh in range(2):
                    wrep = wrepp.tile([128, PIX], bf16, tag="wrep", name=f"wr{gp}_{o}_{gh}")
                    wsrc = bass.AP(
                        tensor=wxd.tensor,
                        offset=wxd.offset + (gh * 64 + o * 8) * PIX,
                        ap=[[0, 16], [PIX, 8], [1, PIX]],
                    )
                    nc.sync.dma_start(out=wrep, in_=wsrc)
                    wr_bc = bass.AP(
                        tensor=wrep.tensor,
                        offset=wrep.offset,
                        ap=[[PIX, 128], [0, 2], [1, PIX]],
                    )
                    for j in range(2):
                        pr2 = prpool.tile(
                            [128, 2, PIX], bf16, tag="pr2", name=f"p2_{gp}_{o}_{gh}_{j}"
                        )
                        nc.vector.tensor_mul(pr2, xts[gh][:, 2 * j:2 * j + 2, :], wr_bc)
                        pr2s[(gh, j)] = pr2
                xout = xobp.tile([128, PIX], bf16, tag="xout", name=f"xo{gp}_{o}")
                if slot_outer:
                    xops = [
                        psum.tile([128, CHN], fp32, tag=f"sp{c}", name=f"xop{gp}_{o}_{c}")
                        for c in range(NCH)
                    ]
                    for s in range(8):
                        gh, cpc = s // 4, s % 4
                        rhs = pr2s[(gh, cpc // 2)]
                        for c in range(NCH):
                            nc.tensor.matmul(
                                xops[c],
                                to_sb[:, s, :],
                                rhs[:, cpc % 2, c * CHN:(c + 1) * CHN],
                                start=(s == 0),
                                stop=(s == 7),
                            )
                    for c in range(NCH):
                        nc.scalar.copy(xout[:, c * CHN:(c + 1) * CHN], xops[c])
                else:
                    for c in range(NCH):
                        xop = psum.tile([128, CHN], fp32, tag="sp", name=f"xop{gp}_{o}_{c}")
                        for s in range(8):
                            gh, cpc = s // 4, s % 4
                            nc.tensor.matmul(
                                xop,
                                to_sb[:, s, :],
                                pr2s[(gh, cpc // 2)][:, cpc % 2, c * CHN:(c + 1) * CHN],
                                start=(s == 0),
                                stop=(s == 7),
                            )
                        nc.scalar.copy(xout[:, c * CHN:(c + 1) * CHN], xop)
                for gh in range(2):
                    # output channels are group-major: c = g*64 + cpc*16 + cpk
                    dst = bass.AP(
                        tensor=out_base.tensor,
                        offset=(o * C + (gp * 2 + gh) * 64) * PIX,
                        ap=[[PIX, 64], [1, PIX]],
                    )
                    nc.sync.dma_start(out=dst, in_=xout[gh * 64:(gh + 1) * 64, :])

            units = [(tt, di) for tt in range(T - 1) for di in (-1, 0, 1)]

            # ---------------- emission schedule ----------------
            with tc.tile_pool(name="frames", bufs=1) as frames, \
                 tc.tile_pool(name="stage", bufs=1) as stage, \
                 tc.tile_pool(name="prods", bufs=2) as prods, \
                 tc.tile_pool(name="pr2", bufs=4) as pr2p, \
                 tc.tile_pool(name="cpsum", bufs=1, space="PSUM") as cpsum, \
                 tc.tile_pool(name="spsum", bufs=1, space="PSUM") as spsum:
                cps = [
                    cpsum.tile([126, CHN], fp32, tag=f"cp{c}", name=f"cps{c}")
                    for c in range(NCH)
                ]
                # half 0 frames + corr
                for t in range(2):
                    load_frame(0, t, frames, stage)
                for k, (tt, di) in enumerate(units):
                    if di == -1 and tt + 2 < T:
                        load_frame(0, tt + 2, frames, stage)
                    corr_unit(0, tt, di, cps, prods)
                corr_drain(0, cps)

                # conv pair 0 + xt loads for its groups
                wxd0 = conv(0, spsum)
                xts0 = [load_xt(0), load_xt(1)]

                # half 1 frames + corr, interleaved with fusion for pair 0
                for t in range(2):
                    load_frame(1, t, frames, stage)
                fuse_at = {2: 0, 5: 1, 7: 2, 10: 3, 13: 4, 15: 5, 18: 6, 20: 7}
                for k, (tt, di) in enumerate(units):
                    if di == -1 and tt + 2 < T:
                        load_frame(1, tt + 2, frames, stage)
                    corr_unit(1, tt, di, cps, prods)
                    if k in fuse_at:
                        fuse_unit(0, fuse_at[k], wxd0, xts0, spsum, pr2p)
                corr_drain(1, cps)

            # pair-1 tail: phase-1 SBUF and corr PSUM banks are free
            with tc.tile_pool(name="pr2b", bufs=6) as pr2b, \
                 tc.tile_pool(name="spsum2", bufs=1, space="PSUM") as spsum2:
                wxd1 = conv(1, spsum2)
                xts1 = [load_xt(2), load_xt(3)]
                for o in range(TO):
                    fuse_unit(1, o, wxd1, xts1, spsum2, pr2b, slot_outer=True)

    nc.compile()
    return nc


def _get_module():
    if "nc" not in _CACHE:
        _CACHE["nc"] = _build_module()
    return _CACHE["nc"]


def _consts(conv_w, conv_b):
    conv_w = np.asarray(conv_w, np.float32)
    conv_b = np.asarray(conv_b, np.float32)
    # block-diagonal fused conv weights per group-pair:
    #   wf2[k, gp, m]; m = gh*64 + o*8 + t; k rows gh*63..+63 hold
    #   conv_w[gp*2+gh, o*8+t, :].  Bias (+1.0 residual when t==o) applied at
    #   the PSUM drain as a per-partition activation bias (bvec).
    wf2 = np.zeros((128, 2, 128), np.float32)
    bvec = np.zeros((128, 2), np.float32)
    for gp in range(2):
        for gh in range(2):
            g = gp * 2 + gh
            half = gh * 63
            for o in range(TO):
                for t in range(T):
                    m = gh * 64 + o * 8 + t
                    wf2[half:half + 63, gp, m] = conv_w[g, o * 8 + t]
                    bvec[m, gp] = conv_b[g, o * 8 + t] + (1.0 if t == o else 0.0)

    # corr-reduce matrix: sliding window puts product r's group-sums into
    # PSUM rows r (channels 0-63) and 63+r (channels 64-127)
    bm = np.zeros((128, 251), np.float32)
    bm[0:64, 125] = 1.0
    bm[64:128, 188] = 1.0

    # t-reduce ones: to[p=(cpk,t), s=(gh,cpc), m] = 1 iff m == gh*64+cpc*16+cpk
    to = np.zeros((128, 8, 128), np.float32)
    for s in range(8):
        gh, cpc = s // 4, s % 4
        for cpk in range(16):
            to[cpk * 8:(cpk + 1) * 8, s, gh * 64 + cpc * 16 + cpk] = 1.0

    return (
        wf2.astype(ml_dtypes.bfloat16),
        bm.astype(ml_dtypes.bfloat16),
        to.astype(ml_dtypes.bfloat16),
        bvec,
    )


def kernel(x, conv_w, conv_b):
    from concourse.bass_utils import run_bass_kernel_spmd

    nc = _get_module()
    wf, bm, to, bv = _consts(conv_w, conv_b)
    x = np.asarray(x, np.float32).astype(ml_dtypes.bfloat16)
    x8 = np.ascontiguousarray(x.reshape(NCORES, T, C, H, W))
    in_maps = [
        {
            "xin": np.ascontiguousarray(x8[i]),
            "wf2": wf,
            "bmat": bm,
            "tones": to,
            "bvec": bv,
        }
        for i in range(NCORES)
    ]
    res = run_bass_kernel_spmd(nc, in_maps, core_ids=list(range(NCORES)))
    outs = [r["out"].astype(np.float32) for r in res.results]
    return np.concatenate(outs, axis=0)


# revision 15
# speedup vs baseline: 1.2728x; 1.0427x over previous
"""CorrelationFusion Trainium2 kernel.

Per-clip math (T=8 frames, G=4 groups, 3x3 correlation window):
  corr[g, tt*9+ij, p] = sum_cp x[tt, g*64+cp, p] * xpad[tt+1, g*64+cp, p+d(ij)]
  wx[g, o*8+t, p]     = sum_i conv_w[g, o*8+t, i]*corr[g, i, p] + conv_b[g, o*8+t]
  out[o, c, p]        = sum_t wx'[g, o*8+t, p] * x[t, c, p],  c = cp*4+g
  (wx' = wx + 1 on the t==o rows -- the residual folded into the conv bias)

Mapping (one clip per NeuronCore, 8 cores data-parallel):
  - per-pixel products on DVE in bf16 (2x mode); channels on partitions
  - partition reductions (over cp / over t) via TensorE matmuls into PSUM
  - single replication-padded frame tile per (half, t); dj shifts read at
    odd element offsets (DVE keeps 2x for unaligned bf16 APs)
  - products batched: 3 dj-shifts per DVE op (corr), 2 cpc per op (fusion)
  - wx replicated to the (cpk,t) layout via one broadcast-read DMA per (g,o)
  - corr(half 1) emission interleaved with fusion(pair 0) to keep DVE+PE busy
  - output stored bf16 on device, upcast to fp32 host-side
"""

import numpy as np
import ml_dtypes

T = 8
TO = 8
G = 4
C = 256
H = 56
W = 56
PIX = H * W
NCORES = 8
PH = 58           # padded rows
PWID = 58         # padded cols
PPITCH = PH * PWID
NCH = 7           # pixel chunks
CHN = 448         # pixels per chunk

_CACHE = {}


def _build_module():
    import concourse.bass as bass
    import concourse.bacc as bacc
    import concourse.mybir as mybir
    import concourse.tile as tile

    fp32 = mybir.dt.float32
    bf16 = mybir.dt.bfloat16

    nc = bacc.Bacc(name="corrfusion")
    xin = nc.dram_tensor("xin", [T, C, H, W], bf16, kind="ExternalInput")
    wf2 = nc.dram_tensor("wf2", [128, 2, 128], bf16, kind="ExternalInput")
    bmat = nc.dram_tensor("bmat", [128, 251], bf16, kind="ExternalInput")
    tones = nc.dram_tensor("tones", [128, 8, 128], bf16, kind="ExternalInput")
    bvec = nc.dram_tensor("bvec", [128, 2], fp32, kind="ExternalInput")
    out = nc.dram_tensor("out", [TO, C, H, W], bf16, kind="ExternalOutput")

    xin_flat = xin.rearrange("t c h w -> t c (h w)")
    xin_base = xin[:, :, :, :]
    out_base = out[:, :, :, :]

    with tile.TileContext(nc) as tc:
        with tc.tile_pool(name="consts", bufs=1) as consts, \
             tc.tile_pool(name="corrbuf", bufs=1) as corrbuf, \
             tc.tile_pool(name="xt", bufs=1) as xtp, \
             tc.tile_pool(name="wrep", bufs=2) as wrepp, \
             tc.tile_pool(name="xob", bufs=1) as xobp, \
             tc.tile_pool(name="wxd", bufs=1, space="DRAM") as wxdp:

            wf_sb = consts.tile([128, 2, 128], bf16)
            nc.sync.dma_start(out=wf_sb, in_=wf2[:, :, :])
            bm_sb = consts.tile([128, 251], bf16)
            nc.sync.dma_start(out=bm_sb, in_=bmat[:, :])
            to_sb = consts.tile([128, 8, 128], bf16)
            nc.sync.dma_start(out=to_sb, in_=tones[:, :, :])
            bv_sb = consts.tile([128, 2], fp32)
            nc.sync.dma_start(out=bv_sb, in_=bvec[:, :])

            corr_sb = [
                corrbuf.tile([128, PIX], bf16, tag=f"corr{i}", name=f"corr{i}")
                for i in range(2)
            ]
            # wx reuses the corr buffer of its pair: conv reads corr chunk c
            # strictly before the drain writes wx chunk c (disjoint regions
            # are tracked, same-region access is read-then-write in order)
            wx_sb = corr_sb
            for i in range(2):
                nc.vector.memset(corr_sb[i][96:128, :], 0.0)

            ptile = {}

            def load_frame(ct, t, frames, stage):
                stg = stage.tile([128, PIX], bf16, tag="fstage", name=f"stg{ct}_{t}")
                nc.sync.dma_start(out=stg, in_=xin_flat[t, ct * 128:(ct + 1) * 128, :])
                stg3 = stg.rearrange("p (h w) -> p h w", h=H)
                P = frames.tile([128, PH, PWID], bf16, tag=f"P{t % 4}", name=f"P{ct}_{t}")
                nc.scalar.copy(P[:, 1:57, 1:57], stg3)
                # replication pads: cols first, then full-width rows (corners ok)
                nc.vector.tensor_copy(P[:, 1:57, 0:1], P[:, 1:57, 1:2])
                nc.vector.tensor_copy(P[:, 1:57, 57:58], P[:, 1:57, 56:57])
                nc.vector.tensor_copy(P[:, 0:1, 0:58], P[:, 1:2, 0:58])
                nc.vector.tensor_copy(P[:, 57:58, 0:58], P[:, 56:57, 0:58])
                ptile[(ct, t)] = P

            def corr_unit(ct, tt, di, cps, prods):
                """One DVE product (3 dj shifts batched) + 21 PE passes."""
                Pa = ptile[(ct, tt)]
                Pb = ptile[(ct, tt + 1)]
                pr = prods.tile([128, 3, PIX], bf16, tag="pr", name=f"pr{ct}_{tt}_{di}")
                a_ap = bass.AP(
                    tensor=Pa.tensor,
                    offset=Pa.offset + 1 * PWID + 1,
                    ap=[[PPITCH, 128], [0, 3], [PWID, 56], [1, 56]],
                )
                b_ap = bass.AP(
                    tensor=Pb.tensor,
                    offset=Pb.offset + (1 + di) * PWID + 0,
                    ap=[[PPITCH, 128], [1, 3], [PWID, 56], [1, 56]],
                )
                pr_view = bass.AP(
                    tensor=pr.tensor,
                    offset=pr.offset,
                    ap=[[3 * PIX, 128], [PIX, 3], [W, 56], [1, 56]],
                )
                nc.vector.tensor_mul(pr_view, a_ap, b_ap)
                for dj in range(3):
                    r = tt * 9 + (di + 1) * 3 + dj
                    lhsT = bm_sb[:, 125 - r:251 - r]
                    for c in range(NCH):
                        nc.tensor.matmul(
                            cps[c],
                            lhsT,
                            pr[:, dj, c * CHN:(c + 1) * CHN],
                            start=(r == 0),
                            stop=(r == 62),
                        )

            def corr_drain(ct, cps):
                for c in range(NCH):
                    nc.scalar.copy(
                        corr_sb[ct][0:126, c * CHN:(c + 1) * CHN], cps[c]
                    )

            def conv(gp, psum):
                """Grouped 1x1 conv: wx = wf2[gp].T @ corr (+bias at drain)."""
                wd = wxdp.tile([128, PIX], bf16, tag=f"wxd{gp}", name=f"wxd{gp}")
                for c in range(NCH):
                    wpp = psum.tile([128, CHN], fp32, tag="sp", name=f"wpp{gp}_{c}")
                    nc.tensor.matmul(
                        wpp,
                        wf_sb[:, gp, :],
                        corr_sb[gp][:, c * CHN:(c + 1) * CHN],
                        start=True,
                        stop=True,
                    )
                    nc.scalar.activation(
                        wx_sb[gp][:, c * CHN:(c + 1) * CHN],
                        wpp,
                        mybir.ActivationFunctionType.Identity,
                        bias=bv_sb[:, gp:gp + 1],
                        scale=1.0,
                    )
                nc.sync.dma_start(out=wd, in_=wx_sb[gp])
                return wd

            def load_xt(g):
                xt = xtp.tile([128, 4, PIX], bf16, tag=f"xt{g % 2}", name=f"xt{g}")
                for cpc in range(4):
                    src = bass.AP(
                        tensor=xin_base.tensor,
                        offset=(cpc * 64 + g) * PIX,
                        ap=[[4 * PIX, 16], [C * PIX, T], [1, PIX]],
                    )
                    nc.sync.dma_start(out=xt[:, cpc, :], in_=src)
                return xt

            def fuse_unit(gp, o, wxd, xts, psum, prpool, slot_outer=False):
                """Weighted frame-sum for one (group-pair, output frame).

                slot_outer=True holds each to_sb weight set across all 7
                chunk passes (needs 7 PSUM banks; only after corr closes)."""
                pr2s = {}
                for gh in range(2):
                    wrep = wrepp.tile([128, PIX], bf16, tag="wrep", name=f"wr{gp}_{o}_{gh}")
                    wsrc = bass.AP(
                        tensor=wxd.tensor,
                        offset=wxd.offset + (gh * 64 + o * 8) * PIX,
                        ap=[[0, 16], [PIX, 8], [1, PIX]],
                    )
                    nc.sync.dma_start(out=wrep, in_=wsrc)
                    wr_bc = bass.AP(
                        tensor=wrep.tensor,
                        offset=wrep.offset,
                        ap=[[PIX, 128], [0, 2], [1, PIX]],
                    )
                    for j in range(2):
                        pr2 = prpool.tile(
                            [128, 2, PIX], bf16, tag="pr2", name=f"p2_{gp}_{o}_{gh}_{j}"
                        )
                        nc.vector.tensor_mul(pr2, xts[gh][:, 2 * j:2 * j + 2, :], wr_bc)
                        pr2s[(gh, j)] = pr2
                xout = xobp.tile([128, PIX], bf16, tag="xout", name=f"xo{gp}_{o}")
                if slot_outer:
                    xops = [
                        psum.tile([128, CHN], fp32, tag=f"sp{c}", name=f"xop{gp}_{o}_{c}")
                        for c in range(NCH)
                    ]
                    for s in range(8):
                        gh, cpc = s // 4, s % 4
                        rhs = pr2s[(gh, cpc // 2)]
                        for c in range(NCH):
                            nc.tensor.matmul(
                                xops[c],
                                to_sb[:, s, :],
                                rhs[:, cpc % 2, c * CHN:(c + 1) * CHN],
                                start=(s == 0),
                                stop=(s == 7),
                            )
                    for c in range(NCH):
                        nc.scalar.copy(xout[:, c * CHN:(c + 1) * CHN], xops[c])
                else:
                    for c in range(NCH):
                        xop = psum.tile([128, CHN], fp32, tag="sp", name=f"xop{gp}_{o}_{c}")
                        for s in range(8):
                            gh, cpc = s // 4, s % 4
                            nc.tensor.matmul(
                                xop,
                                to_sb[:, s, :],
                                pr2s[(gh, cpc // 2)][:, cpc % 2, c * CHN:(c + 1) * CHN],
                                start=(s == 0),
                                stop=(s == 7),
                            )
                        nc.scalar.copy(xout[:, c * CHN:(c + 1) * CHN], xop)
                for gh in range(2):
                    # output channels are group-major: c = g*64 + cpc*16 + cpk
                    dst = bass.AP(
                        tensor=out_base.tensor,
                        offset=(o * C + (gp * 2 + gh) * 64) * PIX,
                        ap=[[PIX, 64], [1, PIX]],
                    )
                    nc.sync.dma_start(out=dst, in_=xout[gh * 64:(gh + 1) * 64, :])

            units = [(tt, di) for tt in range(T - 1) for di in (-1, 0, 1)]

            # ---------------- emission schedule ----------------
            with tc.tile_pool(name="frames", bufs=1) as frames, \
                 tc.tile_pool(name="stage", bufs=2) as stage, \
                 tc.tile_pool(name="prods", bufs=2) as prods, \
                 tc.tile_pool(name="pr2", bufs=4) as pr2p, \
                 tc.tile_pool(name="cpsum", bufs=1, space="PSUM") as cpsum, \
                 tc.tile_pool(name="spsum", bufs=1, space="PSUM") as spsum:
                cps = [
                    cpsum.tile([126, CHN], fp32, tag=f"cp{c}", name=f"cps{c}")
                    for c in range(NCH)
                ]
                # half 0 frames + corr
                for t in range(2):
                    load_frame(0, t, frames, stage)
                for k, (tt, di) in enumerate(units):
                    if di == -1 and tt + 2 < T:
                        load_frame(0, tt + 2, frames, stage)
                    corr_unit(0, tt, di, cps, prods)
                corr_drain(0, cps)

                # conv pair 0 + xt loads for its groups
                wxd0 = conv(0, spsum)
                xts0 = [load_xt(0), load_xt(1)]

                # half 1 frames + corr, interleaved with fusion for pair 0
                for t in range(2):
                    load_frame(1, t, frames, stage)
                fuse_at = {2: 0, 5: 1, 8: 2, 11: 3, 14: 4, 16: 5, 18: 6}
                for k, (tt, di) in enumerate(units):
                    if di == -1 and tt + 2 < T:
                        load_frame(1, tt + 2, frames, stage)
                    corr_unit(1, tt, di, cps, prods)
                    if k in fuse_at:
                        fuse_unit(0, fuse_at[k], wxd0, xts0, spsum, pr2p)
                corr_drain(1, cps)
                wxd1 = conv(1, spsum)
                fuse_unit(0, 7, wxd0, xts0, spsum, pr2p)

            # pair-1 tail: phase-1 SBUF and corr PSUM banks are free
            with tc.tile_pool(name="pr2b", bufs=6) as pr2b, \
                 tc.tile_pool(name="spsum2", bufs=1, space="PSUM") as spsum2:
                xts1 = [load_xt(2), load_xt(3)]
                for o in range(TO):
                    fuse_unit(1, o, wxd1, xts1, spsum2, pr2b, slot_outer=True)

    nc.compile()
    return nc


def _get_module():
    if "nc" not in _CACHE:
        _CACHE["nc"] = _build_module()
    return _CACHE["nc"]


def _consts(conv_w, conv_b):
    conv_w = np.asarray(conv_w, np.float32)
    conv_b = np.asarray(conv_b, np.float32)
    # block-diagonal fused conv weights per group-pair:
    #   wf2[k, gp, m]; m = gh*64 + o*8 + t; k rows gh*63..+63 hold
    #   conv_w[gp*2+gh, o*8+t, :].  Bias (+1.0 residual when t==o) applied at
    #   the PSUM drain as a per-partition activation bias (bvec).
    wf2 = np.zeros((128, 2, 128), np.float32)
    bvec = np.zeros((128, 2), np.float32)
    for gp in range(2):
        for gh in range(2):
            g = gp * 2 + gh
            half = gh * 63
            for o in range(TO):
                for t in range(T):
                    m = gh * 64 + o * 8 + t
                    wf2[half:half + 63, gp, m] = conv_w[g, o * 8 + t]
                    bvec[m, gp] = conv_b[g, o * 8 + t] + (1.0 if t == o else 0.0)

    # corr-reduce matrix: sliding window puts product r's group-sums into
    # PSUM rows r (channels 0-63) and 63+r (channels 64-127)
    bm = np.zeros((128, 251), np.float32)
    bm[0:64, 125] = 1.0
    bm[64:128, 188] = 1.0

    # t-reduce ones: to[p=(cpk,t), s=(gh,cpc), m] = 1 iff m == gh*64+cpc*16+cpk
    to = np.zeros((128, 8, 128), np.float32)
    for s in range(8):
        gh, cpc = s // 4, s % 4
        for cpk in range(16):
            to[cpk * 8:(cpk + 1) * 8, s, gh * 64 + cpc * 16 + cpk] = 1.0

    return (
        wf2.astype(ml_dtypes.bfloat16),
        bm.astype(ml_dtypes.bfloat16),
        to.astype(ml_dtypes.bfloat16),
        bvec,
    )


def kernel(x, conv_w, conv_b):
    from concourse.bass_utils import run_bass_kernel_spmd

    nc = _get_module()
    wf, bm, to, bv = _consts(conv_w, conv_b)
    x = np.asarray(x, np.float32).astype(ml_dtypes.bfloat16)
    x8 = np.ascontiguousarray(x.reshape(NCORES, T, C, H, W))
    in_maps = [
        {
            "xin": np.ascontiguousarray(x8[i]),
            "wf2": wf,
            "bmat": bm,
            "tones": to,
            "bvec": bv,
        }
        for i in range(NCORES)
    ]
    res = run_bass_kernel_spmd(nc, in_maps, core_ids=list(range(NCORES)))
    outs = [r["out"].astype(np.float32) for r in res.results]
    return np.concatenate(outs, axis=0)


# revision 16
# speedup vs baseline: 1.4114x; 1.1089x over previous
"""CorrelationFusion Trainium2 kernel.

Per-clip math (T=8 frames, G=4 groups, 3x3 correlation window):
  corr[g, tt*9+ij, p] = sum_cp x[tt, g*64+cp, p] * xpad[tt+1, g*64+cp, p+d(ij)]
  wx[g, o*8+t, p]     = sum_i conv_w[g, o*8+t, i]*corr[g, i, p] + conv_b[g, o*8+t]
  out[o, c, p]        = sum_t wx'[g, o*8+t, p] * x[t, c, p],  c = cp*4+g
  (wx' = wx + 1 on the t==o rows -- the residual folded into the conv bias)

Mapping (one clip per NeuronCore, 8 cores data-parallel):
  - per-pixel products on DVE in bf16 (2x mode); channels on partitions
  - partition reductions (over cp / over t) via TensorE matmuls into PSUM
  - single replication-padded frame tile per (half, t); dj shifts read at
    odd element offsets (DVE keeps 2x for unaligned bf16 APs)
  - products batched: 3 dj-shifts per DVE op (corr), 2 cpc per op (fusion)
  - wx replicated to the (cpk,t) layout via one broadcast-read DMA per (g,o)
  - corr(half 1) emission interleaved with fusion(pair 0) to keep DVE+PE busy
  - output stored bf16 on device, upcast to fp32 host-side
"""

import numpy as np
import ml_dtypes

T = 8
TO = 8
G = 4
C = 256
H = 56
W = 56
PIX = H * W
NCORES = 8
PH = 58           # padded rows
PWID = 58         # padded cols
PPITCH = PH * PWID
NCH = 7           # pixel chunks
CHN = 448         # pixels per chunk

_CACHE = {}


def _build_module():
    import concourse.bass as bass
    import concourse.bacc as bacc
    import concourse.mybir as mybir
    import concourse.tile as tile

    fp32 = mybir.dt.float32
    bf16 = mybir.dt.bfloat16

    nc = bacc.Bacc(name="corrfusion")
    xin = nc.dram_tensor("xin", [T, C, H, W], bf16, kind="ExternalInput")
    wf2 = nc.dram_tensor("wf2", [128, 2, 128], bf16, kind="ExternalInput")
    bmat = nc.dram_tensor("bmat", [128, 251], bf16, kind="ExternalInput")
    tones = nc.dram_tensor("tones", [128, 8, 128], bf16, kind="ExternalInput")
    bvec = nc.dram_tensor("bvec", [128, 2], fp32, kind="ExternalInput")
    out = nc.dram_tensor("out", [TO, C, H, W], bf16, kind="ExternalOutput")

    xin_flat = xin.rearrange("t c h w -> t c (h w)")
    xin_base = xin[:, :, :, :]
    out_base = out[:, :, :, :]

    with tile.TileContext(nc) as tc:
        with tc.tile_pool(name="consts", bufs=1) as consts, \
             tc.tile_pool(name="corrbuf", bufs=1) as corrbuf, \
             tc.tile_pool(name="xt", bufs=1) as xtp, \
             tc.tile_pool(name="wxd", bufs=1, space="DRAM") as wxdp:

            wf_sb = consts.tile([128, 2, 128], bf16)
            nc.sync.dma_start(out=wf_sb, in_=wf2[:, :, :])
            bm_sb = consts.tile([128, 251], bf16)
            nc.sync.dma_start(out=bm_sb, in_=bmat[:, :])
            to_sb = consts.tile([128, 8, 128], bf16)
            nc.sync.dma_start(out=to_sb, in_=tones[:, :, :])
            bv_sb = consts.tile([128, 2], fp32)
            nc.sync.dma_start(out=bv_sb, in_=bvec[:, :])

            corr_sb = [
                corrbuf.tile([128, PIX], bf16, tag=f"corr{i}", name=f"corr{i}")
                for i in range(2)
            ]
            # wx reuses the corr buffer of its pair: conv reads corr chunk c
            # strictly before the drain writes wx chunk c (disjoint regions
            # are tracked, same-region access is read-then-write in order)
            wx_sb = corr_sb
            for i in range(2):
                nc.vector.memset(corr_sb[i][96:128, :], 0.0)

            ptile = {}

            def load_frame(ct, t, frames, stage):
                stg = stage.tile([128, PIX], bf16, tag="fstage", name=f"stg{ct}_{t}")
                nc.sync.dma_start(out=stg, in_=xin_flat[t, ct * 128:(ct + 1) * 128, :])
                stg3 = stg.rearrange("p (h w) -> p h w", h=H)
                P = frames.tile([128, PH, PWID], bf16, tag=f"P{t % 4}", name=f"P{ct}_{t}")
                nc.scalar.copy(P[:, 1:57, 1:57], stg3)
                # replication pads: cols first, then full-width rows (corners ok)
                nc.vector.tensor_copy(P[:, 1:57, 0:1], P[:, 1:57, 1:2])
                nc.vector.tensor_copy(P[:, 1:57, 57:58], P[:, 1:57, 56:57])
                nc.vector.tensor_copy(P[:, 0:1, 0:58], P[:, 1:2, 0:58])
                nc.vector.tensor_copy(P[:, 57:58, 0:58], P[:, 56:57, 0:58])
                ptile[(ct, t)] = P

            def corr_unit(ct, tt, di, cps, prods):
                """One DVE product (3 dj shifts batched) + 21 PE passes."""
                Pa = ptile[(ct, tt)]
                Pb = ptile[(ct, tt + 1)]
                pr = prods.tile([128, 3, PIX], bf16, tag="pr", name=f"pr{ct}_{tt}_{di}")
                a_ap = bass.AP(
                    tensor=Pa.tensor,
                    offset=Pa.offset + 1 * PWID + 1,
                    ap=[[PPITCH, 128], [0, 3], [PWID, 56], [1, 56]],
                )
                b_ap = bass.AP(
                    tensor=Pb.tensor,
                    offset=Pb.offset + (1 + di) * PWID + 0,
                    ap=[[PPITCH, 128], [1, 3], [PWID, 56], [1, 56]],
                )
                pr_view = bass.AP(
                    tensor=pr.tensor,
                    offset=pr.offset,
                    ap=[[3 * PIX, 128], [PIX, 3], [W, 56], [1, 56]],
                )
                nc.vector.tensor_mul(pr_view, a_ap, b_ap)
                for dj in range(3):
                    r = tt * 9 + (di + 1) * 3 + dj
                    lhsT = bm_sb[:, 125 - r:251 - r]
                    for c in range(NCH):
                        nc.tensor.matmul(
                            cps[c],
                            lhsT,
                            pr[:, dj, c * CHN:(c + 1) * CHN],
                            start=(r == 0),
                            stop=(r == 62),
                        )

            def corr_drain(ct, cps):
                for c in range(NCH):
                    nc.scalar.copy(
                        corr_sb[ct][0:126, c * CHN:(c + 1) * CHN], cps[c]
                    )

            def conv(gp, psum):
                """Grouped 1x1 conv: wx = wf2[gp].T @ corr (+bias at drain)."""
                wd = wxdp.tile([128, PIX], bf16, tag=f"wxd{gp}", name=f"wxd{gp}")
                for c in range(NCH):
                    wpp = psum.tile([128, CHN], fp32, tag="sp", name=f"wpp{gp}_{c}")
                    nc.tensor.matmul(
                        wpp,
                        wf_sb[:, gp, :],
                        corr_sb[gp][:, c * CHN:(c + 1) * CHN],
                        start=True,
                        stop=True,
                    )
                    nc.scalar.activation(
                        wx_sb[gp][:, c * CHN:(c + 1) * CHN],
                        wpp,
                        mybir.ActivationFunctionType.Identity,
                        bias=bv_sb[:, gp:gp + 1],
                        scale=1.0,
                    )
                nc.sync.dma_start(out=wd, in_=wx_sb[gp])
                return wd

            def load_xt(g):
                xt = xtp.tile([128, 4, PIX], bf16, tag=f"xt{g % 2}", name=f"xt{g}")
                for cpc in range(4):
                    src = bass.AP(
                        tensor=xin_base.tensor,
                        offset=(cpc * 64 + g) * PIX,
                        ap=[[4 * PIX, 16], [C * PIX, T], [1, PIX]],
                    )
                    nc.sync.dma_start(out=xt[:, cpc, :], in_=src)
                return xt

            def fuse_unit(gp, o, wxd, xts, psum, prpool, wrepp, xobp, slot_outer=False):
                """Weighted frame-sum for one (group-pair, output frame).

                slot_outer=True holds each to_sb weight set across all 7
                chunk passes (needs 7 PSUM banks; only after corr closes)."""
                pr2s = {}
                for gh in range(2):
                    wrep = wrepp.tile([128, PIX], bf16, tag="wrep", name=f"wr{gp}_{o}_{gh}")
                    wsrc = bass.AP(
                        tensor=wxd.tensor,
                        offset=wxd.offset + (gh * 64 + o * 8) * PIX,
                        ap=[[0, 16], [PIX, 8], [1, PIX]],
                    )
                    nc.sync.dma_start(out=wrep, in_=wsrc)
                    wr_bc = bass.AP(
                        tensor=wrep.tensor,
                        offset=wrep.offset,
                        ap=[[PIX, 128], [0, 2], [1, PIX]],
                    )
                    for j in range(2):
                        pr2 = prpool.tile(
                            [128, 2, PIX], bf16, tag="pr2", name=f"p2_{gp}_{o}_{gh}_{j}"
                        )
                        nc.vector.tensor_mul(pr2, xts[gh][:, 2 * j:2 * j + 2, :], wr_bc)
                        pr2s[(gh, j)] = pr2
                xout = xobp.tile([128, PIX], bf16, tag="xout", name=f"xo{gp}_{o}")
                if slot_outer:
                    xops = [
                        psum.tile([128, CHN], fp32, tag=f"sp{c}", name=f"xop{gp}_{o}_{c}")
                        for c in range(NCH)
                    ]
                    for s in range(8):
                        gh, cpc = s // 4, s % 4
                        rhs = pr2s[(gh, cpc // 2)]
                        for c in range(NCH):
                            nc.tensor.matmul(
                                xops[c],
                                to_sb[:, s, :],
                                rhs[:, cpc % 2, c * CHN:(c + 1) * CHN],
                                start=(s == 0),
                                stop=(s == 7),
                            )
                    for c in range(NCH):
                        nc.scalar.copy(xout[:, c * CHN:(c + 1) * CHN], xops[c])
                else:
                    for c in range(NCH):
                        xop = psum.tile([128, CHN], fp32, tag="sp", name=f"xop{gp}_{o}_{c}")
                        for s in range(8):
                            gh, cpc = s // 4, s % 4
                            nc.tensor.matmul(
                                xop,
                                to_sb[:, s, :],
                                pr2s[(gh, cpc // 2)][:, cpc % 2, c * CHN:(c + 1) * CHN],
                                start=(s == 0),
                                stop=(s == 7),
                            )
                        nc.scalar.copy(xout[:, c * CHN:(c + 1) * CHN], xop)
                for gh in range(2):
                    # output channels are group-major: c = g*64 + cpc*16 + cpk
                    dst = bass.AP(
                        tensor=out_base.tensor,
                        offset=(o * C + (gp * 2 + gh) * 64) * PIX,
                        ap=[[PIX, 64], [1, PIX]],
                    )
                    nc.sync.dma_start(out=dst, in_=xout[gh * 64:(gh + 1) * 64, :])

            units = [(tt, di) for tt in range(T - 1) for di in (-1, 0, 1)]

            # ---------------- emission schedule ----------------
            with tc.tile_pool(name="frames", bufs=1) as frames, \
                 tc.tile_pool(name="stage", bufs=2) as stage, \
                 tc.tile_pool(name="prods", bufs=3) as prods, \
                 tc.tile_pool(name="cpsum", bufs=1, space="PSUM") as cpsum, \
                 tc.tile_pool(name="spsum", bufs=1, space="PSUM") as spsum:
                cps = [
                    cpsum.tile([126, CHN], fp32, tag=f"cp{c}", name=f"cps{c}")
                    for c in range(NCH)
                ]
                # half 0 frames + corr
                for t in range(2):
                    load_frame(0, t, frames, stage)
                for k, (tt, di) in enumerate(units):
                    if di == -1 and tt + 2 < T:
                        load_frame(0, tt + 2, frames, stage)
                    corr_unit(0, tt, di, cps, prods)
                corr_drain(0, cps)

                # conv pair 0 + xt loads for its groups
                wxd0 = conv(0, spsum)
                xts0 = [load_xt(0), load_xt(1)]

                # half 1 frames + corr
                for t in range(2):
                    load_frame(1, t, frames, stage)
                for k, (tt, di) in enumerate(units):
                    if di == -1 and tt + 2 < T:
                        load_frame(1, tt + 2, frames, stage)
                    corr_unit(1, tt, di, cps, prods)
                corr_drain(1, cps)
                wxd1 = conv(1, spsum)

            # fusion for both pairs: phase-1 SBUF and corr PSUM banks are free
            with tc.tile_pool(name="wrep", bufs=3) as wrepp, \
                 tc.tile_pool(name="xob", bufs=2) as xobp, \
                 tc.tile_pool(name="pr2b", bufs=6) as pr2b, \
                 tc.tile_pool(name="spsum2", bufs=1, space="PSUM") as spsum2:
                for o in range(TO):
                    fuse_unit(0, o, wxd0, xts0, spsum2, pr2b, wrepp, xobp, slot_outer=True)
                    if o == 3:
                        xts1 = [load_xt(2), load_xt(3)]
                for o in range(TO):
                    fuse_unit(1, o, wxd1, xts1, spsum2, pr2b, wrepp, xobp, slot_outer=True)

    nc.compile()
    return nc


def _get_module():
    if "nc" not in _CACHE:
        _CACHE["nc"] = _build_module()
    return _CACHE["nc"]


def _consts(conv_w, conv_b):
    conv_w = np.asarray(conv_w, np.float32)
    conv_b = np.asarray(conv_b, np.float32)
    # block-diagonal fused conv weights per group-pair:
    #   wf2[k, gp, m]; m = gh*64 + o*8 + t; k rows gh*63..+63 hold
    #   conv_w[gp*2+gh, o*8+t, :].  Bias (+1.0 residual when t==o) applied at
    #   the PSUM drain as a per-partition activation bias (bvec).
    wf2 = np.zeros((128, 2, 128), np.float32)
    bvec = np.zeros((128, 2), np.float32)
    for gp in range(2):
        for gh in range(2):
            g = gp * 2 + gh
            half = gh * 63
            for o in range(TO):
                for t in range(T):
                    m = gh * 64 + o * 8 + t
                    wf2[half:half + 63, gp, m] = conv_w[g, o * 8 + t]
                    bvec[m, gp] = conv_b[g, o * 8 + t] + (1.0 if t == o else 0.0)

    # corr-reduce matrix: sliding window puts product r's group-sums into
    # PSUM rows r (channels 0-63) and 63+r (channels 64-127)
    bm = np.zeros((128, 251), np.float32)
    bm[0:64, 125] = 1.0
    bm[64:128, 188] = 1.0

    # t-reduce ones: to[p=(cpk,t), s=(gh,cpc), m] = 1 iff m == gh*64+cpc*16+cpk
    to = np.zeros((128, 8, 128), np.float32)
    for s in range(8):
        gh, cpc = s // 4, s % 4
        for cpk in range(16):
            to[cpk * 8:(cpk + 1) * 8, s, gh * 64 + cpc * 16 + cpk] = 1.0

    return (
        wf2.astype(ml_dtypes.bfloat16),
        bm.astype(ml_dtypes.bfloat16),
        to.astype(ml_dtypes.bfloat16),
        bvec,
    )


def kernel(x, conv_w, conv_b):
    from concourse.bass_utils import run_bass_kernel_spmd

    nc = _get_module()
    wf, bm, to, bv = _consts(conv_w, conv_b)
    x = np.asarray(x, np.float32).astype(ml_dtypes.bfloat16)
    x8 = np.ascontiguousarray(x.reshape(NCORES, T, C, H, W))
    in_maps = [
        {
            "xin": np.ascontiguousarray(x8[i]),
            "wf2": wf,
            "bmat": bm,
            "tones": to,
            "bvec": bv,
        }
        for i in range(NCORES)
    ]
    res = run_bass_kernel_spmd(nc, in_maps, core_ids=list(range(NCORES)))
    outs = [r["out"].astype(np.float32) for r in res.results]
    return np.concatenate(outs, axis=0)


# revision 17
# speedup vs baseline: 1.4872x; 1.0537x over previous
"""CorrelationFusion Trainium2 kernel.

Per-clip math (T=8 frames, G=4 groups, 3x3 correlation window):
  corr[g, tt*9+ij, p] = sum_cp x[tt, g*64+cp, p] * xpad[tt+1, g*64+cp, p+d(ij)]
  wx[g, o*8+t, p]     = sum_i conv_w[g, o*8+t, i]*corr[g, i, p] + conv_b[g, o*8+t]
  out[o, c, p]        = sum_t wx'[g, o*8+t, p] * x[t, c, p],  c = cp*4+g
  (wx' = wx + 1 on the t==o rows -- the residual folded into the conv bias)

Mapping (one clip per NeuronCore, 8 cores data-parallel):
  - per-pixel products on DVE in bf16 (2x mode); channels on partitions
  - partition reductions (over cp / over t) via TensorE matmuls into PSUM
  - single replication-padded frame tile per (half, t); dj shifts read at
    odd element offsets (DVE keeps 2x for unaligned bf16 APs)
  - products batched: 3 dj-shifts per DVE op (corr), 2 cpc per op (fusion)
  - wx replicated to the (cpk,t) layout via one broadcast-read DMA per (g,o)
  - corr(half 1) emission interleaved with fusion(pair 0) to keep DVE+PE busy
  - output stored bf16 on device, upcast to fp32 host-side
"""

import numpy as np
import ml_dtypes

T = 8
TO = 8
G = 4
C = 256
H = 56
W = 56
PIX = H * W
NCORES = 8
PH = 58           # padded rows
PWID = 58         # padded cols
PPITCH = PH * PWID
NCH = 7           # pixel chunks
CHN = 448         # pixels per chunk

_CACHE = {}


def _build_module():
    import concourse.bass as bass
    import concourse.bacc as bacc
    import concourse.mybir as mybir
    import concourse.tile as tile

    fp32 = mybir.dt.float32
    bf16 = mybir.dt.bfloat16

    nc = bacc.Bacc(name="corrfusion")
    xin = nc.dram_tensor("xin", [T, C, H, W], bf16, kind="ExternalInput")
    wf2 = nc.dram_tensor("wf2", [128, 2, 128], bf16, kind="ExternalInput")
    bmat = nc.dram_tensor("bmat", [128, 251], bf16, kind="ExternalInput")
    tones = nc.dram_tensor("tones", [128, 8, 128], bf16, kind="ExternalInput")
    bvec = nc.dram_tensor("bvec", [128, 2], fp32, kind="ExternalInput")
    out = nc.dram_tensor("out", [TO, C, H, W], bf16, kind="ExternalOutput")

    xin_flat = xin.rearrange("t c h w -> t c (h w)")
    xin_base = xin[:, :, :, :]
    out_base = out[:, :, :, :]

    with tile.TileContext(nc) as tc:
        with tc.tile_pool(name="consts", bufs=1) as consts, \
             tc.tile_pool(name="corrbuf", bufs=1) as corrbuf, \
             tc.tile_pool(name="xt", bufs=1) as xtp, \
             tc.tile_pool(name="wxd", bufs=1, space="DRAM") as wxdp:

            wf_sb = consts.tile([128, 2, 128], bf16)
            nc.sync.dma_start(out=wf_sb, in_=wf2[:, :, :])
            bm_sb = consts.tile([128, 251], bf16)
            nc.sync.dma_start(out=bm_sb, in_=bmat[:, :])
            to_sb = consts.tile([128, 8, 128], bf16)
            nc.sync.dma_start(out=to_sb, in_=tones[:, :, :])
            bv_sb = consts.tile([128, 2], fp32)
            nc.sync.dma_start(out=bv_sb, in_=bvec[:, :])

            corr_sb = [
                corrbuf.tile([128, PIX], bf16, tag=f"corr{i}", name=f"corr{i}")
                for i in range(2)
            ]
            # wx reuses the corr buffer of its pair: conv reads corr chunk c
            # strictly before the drain writes wx chunk c (disjoint regions
            # are tracked, same-region access is read-then-write in order)
            wx_sb = corr_sb
            for i in range(2):
                nc.vector.memset(corr_sb[i][96:128, :], 0.0)

            ptile = {}

            def load_frame(ct, t, frames, stage):
                stg = stage.tile([128, PIX], bf16, tag="fstage", name=f"stg{ct}_{t}")
                nc.sync.dma_start(out=stg, in_=xin_flat[t, ct * 128:(ct + 1) * 128, :])
                stg3 = stg.rearrange("p (h w) -> p h w", h=H)
                P = frames.tile([128, PH, PWID], bf16, tag=f"P{t % 4}", name=f"P{ct}_{t}")
                nc.scalar.copy(P[:, 1:57, 1:57], stg3)
                # replication pads: cols first, then full-width rows (corners ok)
                nc.vector.tensor_copy(P[:, 1:57, 0:1], P[:, 1:57, 1:2])
                nc.vector.tensor_copy(P[:, 1:57, 57:58], P[:, 1:57, 56:57])
                nc.vector.tensor_copy(P[:, 0:1, 0:58], P[:, 1:2, 0:58])
                nc.vector.tensor_copy(P[:, 57:58, 0:58], P[:, 56:57, 0:58])
                ptile[(ct, t)] = P

            def corr_unit(ct, tt, di, cps, prods):
                """One DVE product (3 dj shifts batched) + 21 PE passes."""
                Pa = ptile[(ct, tt)]
                Pb = ptile[(ct, tt + 1)]
                pr = prods.tile([128, 3, PIX], bf16, tag="pr", name=f"pr{ct}_{tt}_{di}")
                a_ap = bass.AP(
                    tensor=Pa.tensor,
                    offset=Pa.offset + 1 * PWID + 1,
                    ap=[[PPITCH, 128], [0, 3], [PWID, 56], [1, 56]],
                )
                b_ap = bass.AP(
                    tensor=Pb.tensor,
                    offset=Pb.offset + (1 + di) * PWID + 0,
                    ap=[[PPITCH, 128], [1, 3], [PWID, 56], [1, 56]],
                )
                pr_view = bass.AP(
                    tensor=pr.tensor,
                    offset=pr.offset,
                    ap=[[3 * PIX, 128], [PIX, 3], [W, 56], [1, 56]],
                )
                nc.vector.tensor_mul(pr_view, a_ap, b_ap)
                for dj in range(3):
                    r = tt * 9 + (di + 1) * 3 + dj
                    lhsT = bm_sb[:, 125 - r:251 - r]
                    for c in range(NCH):
                        nc.tensor.matmul(
                            cps[c],
                            lhsT,
                            pr[:, dj, c * CHN:(c + 1) * CHN],
                            start=(r == 0),
                            stop=(r == 62),
                        )

            def corr_drain(ct, cps):
                for c in range(NCH):
                    nc.scalar.copy(
                        corr_sb[ct][0:126, c * CHN:(c + 1) * CHN], cps[c]
                    )

            def conv(gp, psum):
                """Grouped 1x1 conv: wx = wf2[gp].T @ corr (+bias at drain)."""
                wd = wxdp.tile([128, PIX], bf16, tag=f"wxd{gp}", name=f"wxd{gp}")
                for c in range(NCH):
                    wpp = psum.tile([128, CHN], fp32, tag="sp", name=f"wpp{gp}_{c}")
                    nc.tensor.matmul(
                        wpp,
                        wf_sb[:, gp, :],
                        corr_sb[gp][:, c * CHN:(c + 1) * CHN],
                        start=True,
                        stop=True,
                    )
                    nc.scalar.activation(
                        wx_sb[gp][:, c * CHN:(c + 1) * CHN],
                        wpp,
                        mybir.ActivationFunctionType.Identity,
                        bias=bv_sb[:, gp:gp + 1],
                        scale=1.0,
                    )
                nc.sync.dma_start(out=wd, in_=wx_sb[gp])
                return wd

            def load_xt(g):
                xt = xtp.tile([128, 4, PIX], bf16, tag=f"xt{g}", name=f"xt{g}")
                for cpc in range(4):
                    src = bass.AP(
                        tensor=xin_base.tensor,
                        offset=(cpc * 64 + g) * PIX,
                        ap=[[4 * PIX, 16], [C * PIX, T], [1, PIX]],
                    )
                    nc.sync.dma_start(out=xt[:, cpc, :], in_=src)
                return xt

            def fuse_unit(gp, o, wxd, xts, psum, prpool, wrepp, xobp, slot_outer=False):
                """Weighted frame-sum for one (group-pair, output frame).

                slot_outer=True holds each to_sb weight set across all 7
                chunk passes (needs 7 PSUM banks; only after corr closes)."""
                pr2s = {}
                for gh in range(2):
                    wrep = wrepp.tile([128, PIX], bf16, tag="wrep", name=f"wr{gp}_{o}_{gh}")
                    wsrc = bass.AP(
                        tensor=wxd.tensor,
                        offset=wxd.offset + (gh * 64 + o * 8) * PIX,
                        ap=[[0, 16], [PIX, 8], [1, PIX]],
                    )
                    nc.sync.dma_start(out=wrep, in_=wsrc)
                    wr_bc = bass.AP(
                        tensor=wrep.tensor,
                        offset=wrep.offset,
                        ap=[[PIX, 128], [0, 2], [1, PIX]],
                    )
                    for j in range(2):
                        pr2 = prpool.tile(
                            [128, 2, PIX], bf16, tag="pr2", name=f"p2_{gp}_{o}_{gh}_{j}"
                        )
                        nc.vector.tensor_mul(pr2, xts[gh][:, 2 * j:2 * j + 2, :], wr_bc)
                        pr2s[(gh, j)] = pr2
                xout = xobp.tile([128, PIX], bf16, tag="xout", name=f"xo{gp}_{o}")
                if slot_outer:
                    xops = [
                        psum.tile([128, CHN], fp32, tag=f"sp{c}", name=f"xop{gp}_{o}_{c}")
                        for c in range(NCH)
                    ]
                    for s in range(8):
                        gh, cpc = s // 4, s % 4
                        rhs = pr2s[(gh, cpc // 2)]
                        for c in range(NCH):
                            nc.tensor.matmul(
                                xops[c],
                                to_sb[:, s, :],
                                rhs[:, cpc % 2, c * CHN:(c + 1) * CHN],
                                start=(s == 0),
                                stop=(s == 7),
                            )
                    for c in range(NCH):
                        nc.scalar.copy(xout[:, c * CHN:(c + 1) * CHN], xops[c])
                else:
                    for c in range(NCH):
                        xop = psum.tile([128, CHN], fp32, tag="sp", name=f"xop{gp}_{o}_{c}")
                        for s in range(8):
                            gh, cpc = s // 4, s % 4
                            nc.tensor.matmul(
                                xop,
                                to_sb[:, s, :],
                                pr2s[(gh, cpc // 2)][:, cpc % 2, c * CHN:(c + 1) * CHN],
                                start=(s == 0),
                                stop=(s == 7),
                            )
                        nc.scalar.copy(xout[:, c * CHN:(c + 1) * CHN], xop)
                for gh in range(2):
                    # output channels are group-major: c = g*64 + cpc*16 + cpk
                    dst = bass.AP(
                        tensor=out_base.tensor,
                        offset=(o * C + (gp * 2 + gh) * 64) * PIX,
                        ap=[[PIX, 64], [1, PIX]],
                    )
                    nc.sync.dma_start(out=dst, in_=xout[gh * 64:(gh + 1) * 64, :])

            units = [(tt, di) for tt in range(T - 1) for di in (-1, 0, 1)]

            # ---------------- emission schedule ----------------
            with tc.tile_pool(name="frames", bufs=1) as frames, \
                 tc.tile_pool(name="stage", bufs=2) as stage, \
                 tc.tile_pool(name="prods", bufs=2) as prods, \
                 tc.tile_pool(name="cpsum", bufs=1, space="PSUM") as cpsum, \
                 tc.tile_pool(name="spsum", bufs=1, space="PSUM") as spsum:
                cps = [
                    cpsum.tile([126, CHN], fp32, tag=f"cp{c}", name=f"cps{c}")
                    for c in range(NCH)
                ]
                # half 0 frames + corr
                for t in range(2):
                    load_frame(0, t, frames, stage)
                for k, (tt, di) in enumerate(units):
                    if di == -1 and tt + 2 < T:
                        load_frame(0, tt + 2, frames, stage)
                    corr_unit(0, tt, di, cps, prods)
                corr_drain(0, cps)

                # conv pair 0 + xt loads (all four groups, fresh buffers)
                wxd0 = conv(0, spsum)
                xts0 = [load_xt(0), load_xt(1)]
                xts1 = None

                # half 1 frames + corr
                for t in range(2):
                    load_frame(1, t, frames, stage)
                for k, (tt, di) in enumerate(units):
                    if di == -1 and tt + 2 < T:
                        load_frame(1, tt + 2, frames, stage)
                    corr_unit(1, tt, di, cps, prods)
                    if k == 8:
                        xts1 = [load_xt(2), load_xt(3)]
                corr_drain(1, cps)
                wxd1 = conv(1, spsum)

            # fusion for both pairs: phase-1 SBUF and corr PSUM banks are free
            with tc.tile_pool(name="wrep", bufs=3) as wrepp, \
                 tc.tile_pool(name="xob", bufs=1) as xobp, \
                 tc.tile_pool(name="pr2b", bufs=5) as pr2b, \
                 tc.tile_pool(name="spsum2", bufs=1, space="PSUM") as spsum2:
                for o in range(TO):
                    fuse_unit(0, o, wxd0, xts0, spsum2, pr2b, wrepp, xobp, slot_outer=True)
                for o in range(TO):
                    fuse_unit(1, o, wxd1, xts1, spsum2, pr2b, wrepp, xobp, slot_outer=True)

    nc.compile()
    return nc


def _get_module():
    if "nc" not in _CACHE:
        _CACHE["nc"] = _build_module()
    return _CACHE["nc"]


def _consts(conv_w, conv_b):
    conv_w = np.asarray(conv_w, np.float32)
    conv_b = np.asarray(conv_b, np.float32)
    # block-diagonal fused conv weights per group-pair:
    #   wf2[k, gp, m]; m = gh*64 + o*8 + t; k rows gh*63..+63 hold
    #   conv_w[gp*2+gh, o*8+t, :].  Bias (+1.0 residual when t==o) applied at
    #   the PSUM drain as a per-partition activation bias (bvec).
    wf2 = np.zeros((128, 2, 128), np.float32)
    bvec = np.zeros((128, 2), np.float32)
    for gp in range(2):
        for gh in range(2):
            g = gp * 2 + gh
            half = gh * 63
            for o in range(TO):
                for t in range(T):
                    m = gh * 64 + o * 8 + t
                    wf2[half:half + 63, gp, m] = conv_w[g, o * 8 + t]
                    bvec[m, gp] = conv_b[g, o * 8 + t] + (1.0 if t == o else 0.0)

    # corr-reduce matrix: sliding window puts product r's group-sums into
    # PSUM rows r (channels 0-63) and 63+r (channels 64-127)
    bm = np.zeros((128, 251), np.float32)
    bm[0:64, 125] = 1.0
    bm[64:128, 188] = 1.0

    # t-reduce ones: to[p=(cpk,t), s=(gh,cpc), m] = 1 iff m == gh*64+cpc*16+cpk
    to = np.zeros((128, 8, 128), np.float32)
    for s in range(8):
        gh, cpc = s // 4, s % 4
        for cpk in range(16):
            to[cpk * 8:(cpk + 1) * 8, s, gh * 64 + cpc * 16 + cpk] = 1.0

    return (
        wf2.astype(ml_dtypes.bfloat16),
        bm.astype(ml_dtypes.bfloat16),
        to.astype(ml_dtypes.bfloat16),
        bvec,
    )


def kernel(x, conv_w, conv_b):
    from concourse.bass_utils import run_bass_kernel_spmd

    nc = _get_module()
    wf, bm, to, bv = _consts(conv_w, conv_b)
    x = np.asarray(x, np.float32).astype(ml_dtypes.bfloat16)
    x8 = np.ascontiguousarray(x.reshape(NCORES, T, C, H, W))
    in_maps = [
        {
            "xin": np.ascontiguousarray(x8[i]),
            "wf2": wf,
            "bmat": bm,
            "tones": to,
            "bvec": bv,
        }
        for i in range(NCORES)
    ]
    res = run_bass_kernel_spmd(nc, in_maps, core_ids=list(range(NCORES)))
    outs = [r["out"].astype(np.float32) for r in res.results]
    return np.concatenate(outs, axis=0)


# revision 18
# speedup vs baseline: 1.5065x; 1.0130x over previous
"""CorrelationFusion Trainium2 kernel.

Per-clip math (T=8 frames, G=4 groups, 3x3 correlation window):
  corr[g, tt*9+ij, p] = sum_cp x[tt, g*64+cp, p] * xpad[tt+1, g*64+cp, p+d(ij)]
  wx[g, o*8+t, p]     = sum_i conv_w[g, o*8+t, i]*corr[g, i, p] + conv_b[g, o*8+t]
  out[o, c, p]        = sum_t wx'[g, o*8+t, p] * x[t, c, p],  c = cp*4+g
  (wx' = wx + 1 on the t==o rows -- the residual folded into the conv bias)

Mapping (one clip per NeuronCore, 8 cores data-parallel):
  - per-pixel products on DVE in bf16 (2x mode); channels on partitions
  - partition reductions (over cp / over t) via TensorE matmuls into PSUM
  - single replication-padded frame tile per (half, t); dj shifts read at
    odd element offsets (DVE keeps 2x for unaligned bf16 APs)
  - products batched: 3 dj-shifts per DVE op (corr), 2 cpc per op (fusion)
  - wx replicated to the (cpk,t) layout via one broadcast-read DMA per (g,o)
  - corr(half 1) emission interleaved with fusion(pair 0) to keep DVE+PE busy
  - output stored bf16 on device, upcast to fp32 host-side
"""

import numpy as np
import ml_dtypes

T = 8
TO = 8
G = 4
C = 256
H = 56
W = 56
PIX = H * W
NCORES = 8
PH = 58           # padded rows
PWID = 58         # padded cols
PPITCH = PH * PWID
NCH = 7           # pixel chunks
CHN = 448         # pixels per chunk

_CACHE = {}


def _build_module():
    import concourse.bass as bass
    import concourse.bacc as bacc
    import concourse.mybir as mybir
    import concourse.tile as tile

    fp32 = mybir.dt.float32
    bf16 = mybir.dt.bfloat16

    nc = bacc.Bacc(name="corrfusion")
    xin = nc.dram_tensor("xin", [T, C, H, W], bf16, kind="ExternalInput")
    wf2 = nc.dram_tensor("wf2", [128, 2, 128], bf16, kind="ExternalInput")
    bmat = nc.dram_tensor("bmat", [128, 251], bf16, kind="ExternalInput")
    tones = nc.dram_tensor("tones", [128, 8, 128], bf16, kind="ExternalInput")
    bvec = nc.dram_tensor("bvec", [128, 2], fp32, kind="ExternalInput")
    out = nc.dram_tensor("out", [TO, C, H, W], bf16, kind="ExternalOutput")

    xin_flat = xin.rearrange("t c h w -> t c (h w)")
    xin_base = xin[:, :, :, :]
    out_base = out[:, :, :, :]

    with tile.TileContext(nc) as tc:
        with tc.tile_pool(name="consts", bufs=1) as consts, \
             tc.tile_pool(name="corrbuf", bufs=1) as corrbuf, \
             tc.tile_pool(name="xt", bufs=1) as xtp, \
             tc.tile_pool(name="wxd", bufs=1, space="DRAM") as wxdp:

            wf_sb = consts.tile([128, 2, 128], bf16)
            nc.sync.dma_start(out=wf_sb, in_=wf2[:, :, :])
            bm_sb = consts.tile([128, 251], bf16)
            nc.sync.dma_start(out=bm_sb, in_=bmat[:, :])
            to_sb = consts.tile([128, 8, 128], bf16)
            nc.sync.dma_start(out=to_sb, in_=tones[:, :, :])
            bv_sb = consts.tile([128, 2], fp32)
            nc.sync.dma_start(out=bv_sb, in_=bvec[:, :])

            corr_sb = [
                corrbuf.tile([128, PIX], bf16, tag=f"corr{i}", name=f"corr{i}")
                for i in range(2)
            ]
            # wx reuses the corr buffer of its pair: conv reads corr chunk c
            # strictly before the drain writes wx chunk c (disjoint regions
            # are tracked, same-region access is read-then-write in order)
            wx_sb = corr_sb
            for i in range(2):
                nc.gpsimd.memset(corr_sb[i][96:128, :], 0.0)

            ptile = {}

            def load_frame(ct, t, frames, stage):
                stg = stage.tile([128, PIX], bf16, tag="fstage", name=f"stg{ct}_{t}")
                nc.sync.dma_start(out=stg, in_=xin_flat[t, ct * 128:(ct + 1) * 128, :])
                stg3 = stg.rearrange("p (h w) -> p h w", h=H)
                P = frames.tile([128, PH, PWID], bf16, tag=f"P{t % 4}", name=f"P{ct}_{t}")
                # first two frames: center copy on DVE (idle during ramp, 4x mode);
                # pads on Scalar everywhere (keeps DVE clear in steady state)
                eng = nc.vector if (ct, t) in ((0, 0), (0, 1)) else nc.scalar
                if eng is nc.vector:
                    eng.tensor_copy(P[:, 1:57, 1:57], stg3)
                else:
                    eng.copy(P[:, 1:57, 1:57], stg3)
                # replication pads: cols first, then full-width rows (corners ok)
                nc.scalar.copy(P[:, 1:57, 0:1], P[:, 1:57, 1:2])
                nc.scalar.copy(P[:, 1:57, 57:58], P[:, 1:57, 56:57])
                nc.scalar.copy(P[:, 0:1, 0:58], P[:, 1:2, 0:58])
                nc.scalar.copy(P[:, 57:58, 0:58], P[:, 56:57, 0:58])
                ptile[(ct, t)] = P

            def corr_unit(ct, tt, di, cps, prods):
                """One DVE product (3 dj shifts batched) + 21 PE passes."""
                Pa = ptile[(ct, tt)]
                Pb = ptile[(ct, tt + 1)]
                pr = prods.tile([128, 3, PIX], bf16, tag="pr", name=f"pr{ct}_{tt}_{di}")
                a_ap = bass.AP(
                    tensor=Pa.tensor,
                    offset=Pa.offset + 1 * PWID + 1,
                    ap=[[PPITCH, 128], [0, 3], [PWID, 56], [1, 56]],
                )
                b_ap = bass.AP(
                    tensor=Pb.tensor,
                    offset=Pb.offset + (1 + di) * PWID + 0,
                    ap=[[PPITCH, 128], [1, 3], [PWID, 56], [1, 56]],
                )
                pr_view = bass.AP(
                    tensor=pr.tensor,
                    offset=pr.offset,
                    ap=[[3 * PIX, 128], [PIX, 3], [W, 56], [1, 56]],
                )
                nc.vector.tensor_mul(pr_view, a_ap, b_ap)
                for dj in range(3):
                    r = tt * 9 + (di + 1) * 3 + dj
                    lhsT = bm_sb[:, 125 - r:251 - r]
                    for c in range(NCH):
                        nc.tensor.matmul(
                            cps[c],
                            lhsT,
                            pr[:, dj, c * CHN:(c + 1) * CHN],
                            start=(r == 0),
                            stop=(r == 62),
                        )

            def corr_drain(ct, cps):
                for c in range(NCH):
                    nc.scalar.copy(
                        corr_sb[ct][0:126, c * CHN:(c + 1) * CHN], cps[c]
                    )

            def conv(gp, psum):
                """Grouped 1x1 conv: wx = wf2[gp].T @ corr (+bias at drain)."""
                wd = wxdp.tile([128, PIX], bf16, tag=f"wxd{gp}", name=f"wxd{gp}")
                for c in range(NCH):
                    wpp = psum.tile([128, CHN], fp32, tag="sp", name=f"wpp{gp}_{c}")
                    nc.tensor.matmul(
                        wpp,
                        wf_sb[:, gp, :],
                        corr_sb[gp][:, c * CHN:(c + 1) * CHN],
                        start=True,
                        stop=True,
                    )
                    nc.scalar.activation(
                        wx_sb[gp][:, c * CHN:(c + 1) * CHN],
                        wpp,
                        mybir.ActivationFunctionType.Identity,
                        bias=bv_sb[:, gp:gp + 1],
                        scale=1.0,
                    )
                nc.sync.dma_start(out=wd, in_=wx_sb[gp])
                return wd

            def load_xt(g):
                xt = xtp.tile([128, 4, PIX], bf16, tag=f"xt{g}", name=f"xt{g}")
                for cpc in range(4):
                    src = bass.AP(
                        tensor=xin_base.tensor,
                        offset=(cpc * 64 + g) * PIX,
                        ap=[[4 * PIX, 16], [C * PIX, T], [1, PIX]],
                    )
                    nc.sync.dma_start(out=xt[:, cpc, :], in_=src)
                return xt

            def fuse_unit(gp, o, wxd, xts, psum, prpool, wrepp, xobp, slot_outer=False):
                """Weighted frame-sum for one (group-pair, output frame).

                slot_outer=True holds each to_sb weight set across all 7
                chunk passes (needs 7 PSUM banks; only after corr closes)."""
                pr2s = {}
                for gh in range(2):
                    wrep = wrepp.tile([128, PIX], bf16, tag="wrep", name=f"wr{gp}_{o}_{gh}")
                    wsrc = bass.AP(
                        tensor=wxd.tensor,
                        offset=wxd.offset + (gh * 64 + o * 8) * PIX,
                        ap=[[0, 16], [PIX, 8], [1, PIX]],
                    )
                    nc.sync.dma_start(out=wrep, in_=wsrc)
                    wr_bc = bass.AP(
                        tensor=wrep.tensor,
                        offset=wrep.offset,
                        ap=[[PIX, 128], [0, 2], [1, PIX]],
                    )
                    for j in range(2):
                        pr2 = prpool.tile(
                            [128, 2, PIX], bf16, tag="pr2", name=f"p2_{gp}_{o}_{gh}_{j}"
                        )
                        nc.vector.tensor_mul(pr2, xts[gh][:, 2 * j:2 * j + 2, :], wr_bc)
                        pr2s[(gh, j)] = pr2
                xout = xobp.tile([128, PIX], bf16, tag="xout", name=f"xo{gp}_{o}")
                if slot_outer:
                    xops = [
                        psum.tile([128, CHN], fp32, tag=f"sp{c}", name=f"xop{gp}_{o}_{c}")
                        for c in range(NCH)
                    ]
                    for s in range(8):
                        gh, cpc = s // 4, s % 4
                        rhs = pr2s[(gh, cpc // 2)]
                        for c in range(NCH):
                            nc.tensor.matmul(
                                xops[c],
                                to_sb[:, s, :],
                                rhs[:, cpc % 2, c * CHN:(c + 1) * CHN],
                                start=(s == 0),
                                stop=(s == 7),
                            )
                    for c in range(NCH):
                        nc.scalar.copy(xout[:, c * CHN:(c + 1) * CHN], xops[c])
                else:
                    for c in range(NCH):
                        xop = psum.tile([128, CHN], fp32, tag="sp", name=f"xop{gp}_{o}_{c}")
                        for s in range(8):
                            gh, cpc = s // 4, s % 4
                            nc.tensor.matmul(
                                xop,
                                to_sb[:, s, :],
                                pr2s[(gh, cpc // 2)][:, cpc % 2, c * CHN:(c + 1) * CHN],
                                start=(s == 0),
                                stop=(s == 7),
                            )
                        nc.scalar.copy(xout[:, c * CHN:(c + 1) * CHN], xop)
                for gh in range(2):
                    # output channels are group-major: c = g*64 + cpc*16 + cpk
                    dst = bass.AP(
                        tensor=out_base.tensor,
                        offset=(o * C + (gp * 2 + gh) * 64) * PIX,
                        ap=[[PIX, 64], [1, PIX]],
                    )
                    nc.sync.dma_start(out=dst, in_=xout[gh * 64:(gh + 1) * 64, :])

            units = [(tt, di) for tt in range(T - 1) for di in (-1, 0, 1)]

            # ---------------- emission schedule ----------------
            with tc.tile_pool(name="frames", bufs=1) as frames, \
                 tc.tile_pool(name="stage", bufs=2) as stage, \
                 tc.tile_pool(name="prods", bufs=2) as prods, \
                 tc.tile_pool(name="cpsum", bufs=1, space="PSUM") as cpsum, \
                 tc.tile_pool(name="spsum", bufs=1, space="PSUM") as spsum:
                cps = [
                    cpsum.tile([126, CHN], fp32, tag=f"cp{c}", name=f"cps{c}")
                    for c in range(NCH)
                ]
                # half 0 frames + corr
                for t in range(2):
                    load_frame(0, t, frames, stage)
                for k, (tt, di) in enumerate(units):
                    if di == -1 and tt + 2 < T:
                        load_frame(0, tt + 2, frames, stage)
                    corr_unit(0, tt, di, cps, prods)
                corr_drain(0, cps)

                # conv pair 0 + xt loads (all four groups, fresh buffers)
                wxd0 = conv(0, spsum)
                xts0 = [load_xt(0), load_xt(1)]
                xts1 = None

                # half 1 frames + corr
                for t in range(2):
                    load_frame(1, t, frames, stage)
                for k, (tt, di) in enumerate(units):
                    if di == -1 and tt + 2 < T:
                        load_frame(1, tt + 2, frames, stage)
                    corr_unit(1, tt, di, cps, prods)
                    if k == 8:
                        xts1 = [load_xt(2), load_xt(3)]
                corr_drain(1, cps)
                wxd1 = conv(1, spsum)

            # fusion for both pairs: phase-1 SBUF and corr PSUM banks are free
            with tc.tile_pool(name="wrep", bufs=3) as wrepp, \
                 tc.tile_pool(name="xob", bufs=1) as xobp, \
                 tc.tile_pool(name="pr2b", bufs=5) as pr2b, \
                 tc.tile_pool(name="spsum2", bufs=1, space="PSUM") as spsum2:
                for o in range(TO):
                    fuse_unit(0, o, wxd0, xts0, spsum2, pr2b, wrepp, xobp, slot_outer=True)
                for o in range(TO):
                    fuse_unit(1, o, wxd1, xts1, spsum2, pr2b, wrepp, xobp, slot_outer=True)

    nc.compile()
    return nc


def _get_module():
    if "nc" not in _CACHE:
        _CACHE["nc"] = _build_module()
    return _CACHE["nc"]


def _consts(conv_w, conv_b):
    conv_w = np.asarray(conv_w, np.float32)
    conv_b = np.asarray(conv_b, np.float32)
    # block-diagonal fused conv weights per group-pair:
    #   wf2[k, gp, m]; m = gh*64 + o*8 + t; k rows gh*63..+63 hold
    #   conv_w[gp*2+gh, o*8+t, :].  Bias (+1.0 residual when t==o) applied at
    #   the PSUM drain as a per-partition activation bias (bvec).
    wf2 = np.zeros((128, 2, 128), np.float32)
    bvec = np.zeros((128, 2), np.float32)
    for gp in range(2):
        for gh in range(2):
            g = gp * 2 + gh
            half = gh * 63
            for o in range(TO):
                for t in range(T):
                    m = gh * 64 + o * 8 + t
                    wf2[half:half + 63, gp, m] = conv_w[g, o * 8 + t]
                    bvec[m, gp] = conv_b[g, o * 8 + t] + (1.0 if t == o else 0.0)

    # corr-reduce matrix: sliding window puts product r's group-sums into
    # PSUM rows r (channels 0-63) and 63+r (channels 64-127)
    bm = np.zeros((128, 251), np.float32)
    bm[0:64, 125] = 1.0
    bm[64:128, 188] = 1.0

    # t-reduce ones: to[p=(cpk,t), s=(gh,cpc), m] = 1 iff m == gh*64+cpc*16+cpk
    to = np.zeros((128, 8, 128), np.float32)
    for s in range(8):
        gh, cpc = s // 4, s % 4
        for cpk in range(16):
            to[cpk * 8:(cpk + 1) * 8, s, gh * 64 + cpc * 16 + cpk] = 1.0

    return (
        wf2.astype(ml_dtypes.bfloat16),
        bm.astype(ml_dtypes.bfloat16),
        to.astype(ml_dtypes.bfloat16),
        bvec,
    )


def kernel(x, conv_w, conv_b):
    from concourse.bass_utils import run_bass_kernel_spmd

    nc = _get_module()
    wf, bm, to, bv = _consts(conv_w, conv_b)
    x = np.asarray(x, np.float32).astype(ml_dtypes.bfloat16)
    x8 = np.ascontiguousarray(x.reshape(NCORES, T, C, H, W))
    in_maps = [
        {
            "xin": np.ascontiguousarray(x8[i]),
            "wf2": wf,
            "bmat": bm,
            "tones": to,
            "bvec": bv,
        }
        for i in range(NCORES)
    ]
    res = run_bass_kernel_spmd(nc, in_maps, core_ids=list(range(NCORES)))
    outs = [r["out"].astype(np.float32) for r in res.results]
    return np.concatenate(outs, axis=0)


# revision 19
# speedup vs baseline: 1.5155x; 1.0060x over previous
"""CorrelationFusion Trainium2 kernel.

Per-clip math (T=8 frames, G=4 groups, 3x3 correlation window):
  corr[g, tt*9+ij, p] = sum_cp x[tt, g*64+cp, p] * xpad[tt+1, g*64+cp, p+d(ij)]
  wx[g, o*8+t, p]     = sum_i conv_w[g, o*8+t, i]*corr[g, i, p] + conv_b[g, o*8+t]
  out[o, c, p]        = sum_t wx'[g, o*8+t, p] * x[t, c, p],  c = cp*4+g
  (wx' = wx + 1 on the t==o rows -- the residual folded into the conv bias)

Mapping (one clip per NeuronCore, 8 cores data-parallel):
  - per-pixel products on DVE in bf16 (2x mode); channels on partitions
  - partition reductions (over cp / over t) via TensorE matmuls into PSUM
  - single replication-padded frame tile per (half, t); dj shifts read at
    odd element offsets (DVE keeps 2x for unaligned bf16 APs)
  - products batched: 3 dj-shifts per DVE op (corr), 2 cpc per op (fusion)
  - wx replicated to the (cpk,t) layout via one broadcast-read DMA per (g,o)
  - corr(half 1) emission interleaved with fusion(pair 0) to keep DVE+PE busy
  - output stored bf16 on device, upcast to fp32 host-side
"""

import numpy as np
import ml_dtypes

T = 8
TO = 8
G = 4
C = 256
H = 56
W = 56
PIX = H * W
NCORES = 8
PH = 58           # padded rows
PWID = 58         # padded cols
PPITCH = PH * PWID
NCH = 7           # pixel chunks
CHN = 448         # pixels per chunk

_CACHE = {}


def _build_module():
    import concourse.bass as bass
    import concourse.bacc as bacc
    import concourse.mybir as mybir
    import concourse.tile as tile

    fp32 = mybir.dt.float32
    bf16 = mybir.dt.bfloat16

    nc = bacc.Bacc(name="corrfusion")
    xin = nc.dram_tensor("xin", [T, C, H, W], bf16, kind="ExternalInput")
    wf2 = nc.dram_tensor("wf2", [128, 2, 128], bf16, kind="ExternalInput")
    bmat = nc.dram_tensor("bmat", [128, 251], bf16, kind="ExternalInput")
    tones = nc.dram_tensor("tones", [128, 8, 128], bf16, kind="ExternalInput")
    bvec = nc.dram_tensor("bvec", [128, 2], fp32, kind="ExternalInput")
    out = nc.dram_tensor("out", [TO, C, H, W], bf16, kind="ExternalOutput")

    xin_flat = xin.rearrange("t c h w -> t c (h w)")
    xin_base = xin[:, :, :, :]
    out_base = out[:, :, :, :]

    with tile.TileContext(nc) as tc:
        with tc.tile_pool(name="consts", bufs=1) as consts, \
             tc.tile_pool(name="corrbuf", bufs=1) as corrbuf, \
             tc.tile_pool(name="xt", bufs=1) as xtp, \
             tc.tile_pool(name="wxd", bufs=1, space="DRAM") as wxdp:

            wf_sb = consts.tile([128, 2, 128], bf16)
            nc.sync.dma_start(out=wf_sb, in_=wf2[:, :, :])
            bm_sb = consts.tile([128, 251], bf16)
            nc.sync.dma_start(out=bm_sb, in_=bmat[:, :])
            to_sb = consts.tile([128, 8, 128], bf16)
            nc.sync.dma_start(out=to_sb, in_=tones[:, :, :])
            bv_sb = consts.tile([128, 2], fp32)
            nc.sync.dma_start(out=bv_sb, in_=bvec[:, :])

            corr_sb = [
                corrbuf.tile([128, PIX], bf16, tag=f"corr{i}", name=f"corr{i}")
                for i in range(2)
            ]
            # wx reuses the corr buffer of its pair: conv reads corr chunk c
            # strictly before the drain writes wx chunk c (disjoint regions
            # are tracked, same-region access is read-then-write in order)
            wx_sb = corr_sb
            for i in range(2):
                nc.gpsimd.memset(corr_sb[i][96:128, :], 0.0)

            ptile = {}

            def load_frame(ct, t, frames, stage):
                stg = stage.tile([128, PIX], bf16, tag="fstage", name=f"stg{ct}_{t}")
                nc.sync.dma_start(out=stg, in_=xin_flat[t, ct * 128:(ct + 1) * 128, :])
                stg3 = stg.rearrange("p (h w) -> p h w", h=H)
                P = frames.tile([128, PH, PWID], bf16, tag=f"P{t % 4}", name=f"P{ct}_{t}")
                # first two frames: center copy on DVE (idle during ramp, 4x mode);
                # pads on Scalar everywhere (keeps DVE clear in steady state)
                eng = nc.vector if (ct, t) in ((0, 0), (0, 1)) else nc.scalar
                if eng is nc.vector:
                    eng.tensor_copy(P[:, 1:57, 1:57], stg3)
                else:
                    eng.copy(P[:, 1:57, 1:57], stg3)
                # replication pads: cols first, then full-width rows (corners ok)
                nc.scalar.copy(P[:, 1:57, 0:1], P[:, 1:57, 1:2])
                nc.scalar.copy(P[:, 1:57, 57:58], P[:, 1:57, 56:57])
                nc.scalar.copy(P[:, 0:1, 0:58], P[:, 1:2, 0:58])
                nc.scalar.copy(P[:, 57:58, 0:58], P[:, 56:57, 0:58])
                ptile[(ct, t)] = P

            def corr_unit(ct, tt, di, cps, prods):
                """One DVE product (3 dj shifts batched) + 21 PE passes."""
                Pa = ptile[(ct, tt)]
                Pb = ptile[(ct, tt + 1)]
                pr = prods.tile([128, 3, PIX], bf16, tag="pr", name=f"pr{ct}_{tt}_{di}")
                a_ap = bass.AP(
                    tensor=Pa.tensor,
                    offset=Pa.offset + 1 * PWID + 1,
                    ap=[[PPITCH, 128], [0, 3], [PWID, 56], [1, 56]],
                )
                b_ap = bass.AP(
                    tensor=Pb.tensor,
                    offset=Pb.offset + (1 + di) * PWID + 0,
                    ap=[[PPITCH, 128], [1, 3], [PWID, 56], [1, 56]],
                )
                pr_view = bass.AP(
                    tensor=pr.tensor,
                    offset=pr.offset,
                    ap=[[3 * PIX, 128], [PIX, 3], [W, 56], [1, 56]],
                )
                nc.vector.tensor_mul(pr_view, a_ap, b_ap)
                for dj in range(3):
                    r = tt * 9 + (di + 1) * 3 + dj
                    lhsT = bm_sb[:, 125 - r:251 - r]
                    for c in range(NCH):
                        nc.tensor.matmul(
                            cps[c],
                            lhsT,
                            pr[:, dj, c * CHN:(c + 1) * CHN],
                            start=(r == 0),
                            stop=(r == 62),
                        )

            def corr_drain(ct, cps):
                for c in range(NCH):
                    nc.scalar.copy(
                        corr_sb[ct][0:126, c * CHN:(c + 1) * CHN], cps[c]
                    )

            def conv(gp, psum):
                """Grouped 1x1 conv: wx = wf2[gp].T @ corr (+bias at drain)."""
                wd = wxdp.tile([128, PIX], bf16, tag=f"wxd{gp}", name=f"wxd{gp}")
                for c in range(NCH):
                    wpp = psum.tile([128, CHN], fp32, tag="sp", name=f"wpp{gp}_{c}")
                    nc.tensor.matmul(
                        wpp,
                        wf_sb[:, gp, :],
                        corr_sb[gp][:, c * CHN:(c + 1) * CHN],
                        start=True,
                        stop=True,
                    )
                    nc.scalar.activation(
                        wx_sb[gp][:, c * CHN:(c + 1) * CHN],
                        wpp,
                        mybir.ActivationFunctionType.Identity,
                        bias=bv_sb[:, gp:gp + 1],
                        scale=1.0,
                    )
                nc.sync.dma_start(out=wd, in_=wx_sb[gp])
                return wd

            def load_xt(g):
                xt = xtp.tile([128, 4, PIX], bf16, tag=f"xt{g}", name=f"xt{g}")
                for cpc in range(4):
                    src = bass.AP(
                        tensor=xin_base.tensor,
                        offset=(cpc * 64 + g) * PIX,
                        ap=[[4 * PIX, 16], [C * PIX, T], [1, PIX]],
                    )
                    nc.sync.dma_start(out=xt[:, cpc, :], in_=src)
                return xt

            def fuse_unit(gp, o, wxd, xts, psum, prpool, wrepp, xobp, slot_outer=False):
                """Weighted frame-sum for one (group-pair, output frame).

                slot_outer=True holds each to_sb weight set across all 7
                chunk passes (needs 7 PSUM banks; only after corr closes)."""
                pr2s = {}
                for gh in range(2):
                    wrep = wrepp.tile([128, PIX], bf16, tag="wrep", name=f"wr{gp}_{o}_{gh}")
                    wsrc = bass.AP(
                        tensor=wxd.tensor,
                        offset=wxd.offset + (gh * 64 + o * 8) * PIX,
                        ap=[[0, 16], [PIX, 8], [1, PIX]],
                    )
                    nc.sync.dma_start(out=wrep, in_=wsrc)
                    wr_bc = bass.AP(
                        tensor=wrep.tensor,
                        offset=wrep.offset,
                        ap=[[PIX, 128], [0, 2], [1, PIX]],
                    )
                    for j in range(2):
                        pr2 = prpool.tile(
                            [128, 2, PIX], bf16, tag="pr2", name=f"p2_{gp}_{o}_{gh}_{j}"
                        )
                        nc.vector.tensor_mul(pr2, xts[gh][:, 2 * j:2 * j + 2, :], wr_bc)
                        pr2s[(gh, j)] = pr2
                xout = xobp.tile([128, PIX], bf16, tag="xout", name=f"xo{gp}_{o}")
                if slot_outer:
                    xops = [
                        psum.tile([128, CHN], fp32, tag=f"sp{c}", name=f"xop{gp}_{o}_{c}")
                        for c in range(NCH)
                    ]
                    for s in range(8):
                        gh, cpc = s // 4, s % 4
                        rhs = pr2s[(gh, cpc // 2)]
                        for c in range(NCH):
                            nc.tensor.matmul(
                                xops[c],
                                to_sb[:, s, :],
                                rhs[:, cpc % 2, c * CHN:(c + 1) * CHN],
                                start=(s == 0),
                                stop=(s == 7),
                            )
                    for c in range(NCH):
                        nc.scalar.copy(xout[:, c * CHN:(c + 1) * CHN], xops[c])
                else:
                    for c in range(NCH):
                        xop = psum.tile([128, CHN], fp32, tag="sp", name=f"xop{gp}_{o}_{c}")
                        for s in range(8):
                            gh, cpc = s // 4, s % 4
                            nc.tensor.matmul(
                                xop,
                                to_sb[:, s, :],
                                pr2s[(gh, cpc // 2)][:, cpc % 2, c * CHN:(c + 1) * CHN],
                                start=(s == 0),
                                stop=(s == 7),
                            )
                        nc.scalar.copy(xout[:, c * CHN:(c + 1) * CHN], xop)
                for gh in range(2):
                    # output channels are group-major: c = g*64 + cpc*16 + cpk
                    dst = bass.AP(
                        tensor=out_base.tensor,
                        offset=(o * C + (gp * 2 + gh) * 64) * PIX,
                        ap=[[PIX, 64], [1, PIX]],
                    )
                    nc.sync.dma_start(out=dst, in_=xout[gh * 64:(gh + 1) * 64, :])

            units = [(tt, di) for tt in range(T - 1) for di in (-1, 0, 1)]

            # ---------------- emission schedule ----------------
            with tc.tile_pool(name="frames", bufs=1) as frames, \
                 tc.tile_pool(name="stage", bufs=2) as stage, \
                 tc.tile_pool(name="prods", bufs=3) as prods, \
                 tc.tile_pool(name="cpsum", bufs=1, space="PSUM") as cpsum, \
                 tc.tile_pool(name="spsum", bufs=1, space="PSUM") as spsum:
                cps = [
                    cpsum.tile([126, CHN], fp32, tag=f"cp{c}", name=f"cps{c}")
                    for c in range(NCH)
                ]
                # half 0 frames + corr
                for t in range(2):
                    load_frame(0, t, frames, stage)
                for k, (tt, di) in enumerate(units):
                    if di == -1 and tt + 2 < T:
                        load_frame(0, tt + 2, frames, stage)
                    corr_unit(0, tt, di, cps, prods)
                corr_drain(0, cps)

                # conv pair 0 + xt loads (all four groups, fresh buffers)
                wxd0 = conv(0, spsum)
                xts0 = [load_xt(0), load_xt(1)]
                xts1 = None

                # half 1 frames + corr
                for t in range(2):
                    load_frame(1, t, frames, stage)
                for k, (tt, di) in enumerate(units):
                    if di == -1 and tt + 2 < T:
                        load_frame(1, tt + 2, frames, stage)
                    corr_unit(1, tt, di, cps, prods)
                    if k == 8:
                        xts1 = [load_xt(2), load_xt(3)]
                corr_drain(1, cps)
                wxd1 = conv(1, spsum)

            # fusion for both pairs: phase-1 SBUF and corr PSUM banks are free
            with tc.tile_pool(name="wrep", bufs=3) as wrepp, \
                 tc.tile_pool(name="xob", bufs=1) as xobp, \
                 tc.tile_pool(name="pr2b", bufs=5) as pr2b, \
                 tc.tile_pool(name="spsum2", bufs=1, space="PSUM") as spsum2:
                for o in range(TO):
                    fuse_unit(0, o, wxd0, xts0, spsum2, pr2b, wrepp, xobp, slot_outer=True)
                for o in range(TO):
                    fuse_unit(1, o, wxd1, xts1, spsum2, pr2b, wrepp, xobp, slot_outer=True)

    nc.compile()
    return nc


def _get_module():
    if "nc" not in _CACHE:
        _CACHE["nc"] = _build_module()
    return _CACHE["nc"]


def _consts(conv_w, conv_b):
    conv_w = np.asarray(conv_w, np.float32)
    conv_b = np.asarray(conv_b, np.float32)
    # block-diagonal fused conv weights per group-pair:
    #   wf2[k, gp, m]; m = gh*64 + o*8 + t; k rows gh*63..+63 hold
    #   conv_w[gp*2+gh, o*8+t, :].  Bias (+1.0 residual when t==o) applied at
    #   the PSUM drain as a per-partition activation bias (bvec).
    wf2 = np.zeros((128, 2, 128), np.float32)
    bvec = np.zeros((128, 2), np.float32)
    for gp in range(2):
        for gh in range(2):
            g = gp * 2 + gh
            half = gh * 63
            for o in range(TO):
                for t in range(T):
                    m = gh * 64 + o * 8 + t
                    wf2[half:half + 63, gp, m] = conv_w[g, o * 8 + t]
                    bvec[m, gp] = conv_b[g, o * 8 + t] + (1.0 if t == o else 0.0)

    # corr-reduce matrix: sliding window puts product r's group-sums into
    # PSUM rows r (channels 0-63) and 63+r (channels 64-127)
    bm = np.zeros((128, 251), np.float32)
    bm[0:64, 125] = 1.0
    bm[64:128, 188] = 1.0

    # t-reduce ones: to[p=(cpk,t), s=(gh,cpc), m] = 1 iff m == gh*64+cpc*16+cpk
    to = np.zeros((128, 8, 128), np.float32)
    for s in range(8):
        gh, cpc = s // 4, s % 4
        for cpk in range(16):
            to[cpk * 8:(cpk + 1) * 8, s, gh * 64 + cpc * 16 + cpk] = 1.0

    return (
        wf2.astype(ml_dtypes.bfloat16),
        bm.astype(ml_dtypes.bfloat16),
        to.astype(ml_dtypes.bfloat16),
        bvec,
    )


def kernel(x, conv_w, conv_b):
    from concourse.bass_utils import run_bass_kernel_spmd

    nc = _get_module()
    wf, bm, to, bv = _consts(conv_w, conv_b)
    x = np.asarray(x, np.float32).astype(ml_dtypes.bfloat16)
    x8 = np.ascontiguousarray(x.reshape(NCORES, T, C, H, W))
    in_maps = [
        {
            "xin": np.ascontiguousarray(x8[i]),
            "wf2": wf,
            "bmat": bm,
            "tones": to,
            "bvec": bv,
        }
        for i in range(NCORES)
    ]
    res = run_bass_kernel_spmd(nc, in_maps, core_ids=list(range(NCORES)))
    outs = [r["out"].astype(np.float32) for r in res.results]
    return np.concatenate(outs, axis=0)
